# revision 1
# baseline (speedup 1.0000x reference)
"""Trainium2 Bass kernel for nn_CachedDecoderOnly (cached decode step).

Strategy (8 NeuronCores, SPMD — same NEFF, per-core data):
  - Only layer 0 touches the full sequence. Its attention is algebraically
    folded:   scores = (out+pe) @ (Q @ wk_head)^T,   ctx = softmax @ (out+pe),
    u = ctx @ wv^T (per head), so the big K/V projections (137 GFLOP) vanish.
  - Sequence (S=4096) is sharded over the 8 cores (512 rows x 8 batches each);
    partial softmax (unnormalized exp sums, scores are O(4) so no max needed)
    is combined with one AllGather + on-device rank-sum.
  - Layers 1..5 collapse (softmax over a single key == 1) to
    x -> wv -> wo -> LN -> FFN -> LN; they run tensor-parallel over the 8
    cores in a feature-major [d_partition, batch_free] layout with one
    AllGather + rank-sum per matmul pair, LN via PE stat-matmuls.
  - Weights are streamed as bf16 (fp32 accumulation in PSUM); activations在
    attention are bf16, everything LN/residual is fp32.
"""

import numpy as np

B, S, D, H, DH, FF, V, L = 8, 4096, 1024, 16, 64, 2048, 256, 6
NCORE = 8
SC = S // NCORE          # 512 sequence rows per core per batch
NT = SC // 128           # 4 s-tiles
ND = D // 128            # 8 d-tiles
SCALE = 1.0 / 8.0        # 1/sqrt(DH)

_BUILT = {}


def _pe_np():
    pos = np.arange(S, dtype=np.float32)[:, None]
    div = np.exp(np.arange(0, D, 2, dtype=np.float32) * (-np.log(10000.0) / D))
    pe = np.zeros((S, D), np.float32)
    pe[:, 0::2] = np.sin(pos * div)
    pe[:, 1::2] = np.cos(pos * div)
    return pe


class _Smalls:
    """Packs per-core [128, n] fp32 constant columns; returns column slices."""

    def __init__(self):
        self.cols = {}
        self.n = 0

    def add(self, name, arr):  # arr [128, k]
        arr = np.asarray(arr, np.float32)
        assert arr.shape[0] == 128
        k = arr.shape[1] if arr.ndim == 2 else 1
        arr = arr.reshape(128, k)
        self.cols[name] = (self.n, k, arr)
        self.n += k

    def pack(self):
        out = np.zeros((128, max(self.n, 1)), np.float32)
        for off, k, arr in self.cols.values():
            out[:, off:off + k] = arr
        return out

    def sl(self, name):
        off, k, _ = self.cols[name]
        return off, k


def _vec_T(v):
    """[D] fp32 -> [128, ND] feature-major tile layout: out[p, dt] = v[dt*128+p]."""
    return np.ascontiguousarray(v.reshape(ND, 128).T)


def _prep_inputs(inputs):
    """Host-side weight re-layout + sharding. Returns (in_maps, flags, meta)."""
    f32 = np.float32
    g = {k: np.asarray(v) for k, v in inputs.items()}
    pe = _pe_np()

    in_proj_w = g['in_proj_w'].astype(f32)
    in_proj_b = g['in_proj_b'].astype(f32)
    out_proj_w = g['out_proj_w'].astype(f32)
    out_proj_b = g['out_proj_b'].astype(f32)
    lin1_w, lin1_b = g['lin1_w'].astype(f32), g['lin1_b'].astype(f32)
    lin2_w, lin2_b = g['lin2_w'].astype(f32), g['lin2_b'].astype(f32)
    ln1_w, ln1_b = g['ln1_w'].astype(f32), g['ln1_b'].astype(f32)
    ln2_w, ln2_b = g['ln2_w'].astype(f32), g['ln2_b'].astype(f32)
    w_out, b_out = g['w_out'].astype(f32), g['b_out'].astype(f32)
    tgt = g['tgt'].astype(f32)

    flags = {
        'use_bq': bool(np.any(in_proj_b[0, :D] != 0)),
        'use_bv': bool(np.any(in_proj_b[:, 2 * D:] != 0)),
        'use_bo': bool(np.any(out_proj_b != 0)),
        'use_b1': bool(np.any(lin1_b != 0)),
        'use_b2': bool(np.any(lin2_b != 0)),
        'use_bout': bool(np.any(b_out != 0)),
        'use_ln1': bool(np.any(ln1_w != 1) or np.any(ln1_b != 0)),
        'use_ln2': bool(np.any(ln2_w != 1) or np.any(ln2_b != 0)),
    }

    import ml_dtypes
    bf16 = ml_dtypes.bfloat16

    wq = in_proj_w[0, 0:D]
    wk = in_proj_w[0, D:2 * D]

    q = (tgt[:, -1, :] + pe[-1]).astype(f32)             # [B, D]
    qT = np.ascontiguousarray(q.T.reshape(ND, 128, B).transpose(1, 0, 2))  # [128, ND, B]

    shared = {
        'qT': qT,
        'wqT': np.ascontiguousarray(wq.T.reshape(ND, 128, D)).astype(bf16),
        'wkn': np.ascontiguousarray(wk.reshape(8, 128, D)).astype(bf16),
        'wvT0': np.ascontiguousarray(in_proj_w[0, 2 * D:].T.reshape(ND, 128, D)).astype(bf16),
    }
    # per-core maps
    in_maps = []
    for c in range(NCORE):
        m = dict(shared)
        m['tgtC'] = np.ascontiguousarray(
            tgt[:, c * SC:(c + 1) * SC, :].reshape(B, NT, 128, D))
        m['peC'] = np.ascontiguousarray(
            pe[c * SC:(c + 1) * SC].reshape(NT, 128, D))
        # layer-0 out_proj: per-head blocks, j-slice for this core
        # woT0 [16, 64, 128]: [h, dh, jl] = wo0[c*128+jl, h*64+dh]
        m['woT0'] = np.ascontiguousarray(
            out_proj_w[0].T[:, c * 128:(c + 1) * 128].reshape(H, DH, 128)).astype(bf16)
        for i in range(L):
            w1T = lin1_w[i].T[:, c * 256:(c + 1) * 256]          # [1024, 256]
            m[f'w1T{i}'] = np.ascontiguousarray(
                w1T.reshape(ND, 128, 2, 128).transpose(2, 0, 1, 3)).astype(bf16)
            w2T = lin2_w[i].T[c * 256:(c + 1) * 256, :]          # [256, 1024]
            m[f'w2T{i}'] = np.ascontiguousarray(
                w2T.reshape(2, 128, ND, 128).transpose(0, 2, 1, 3)).astype(bf16)
        for i in range(1, L):
            wv = in_proj_w[i, 2 * D:]
            m[f'wvT{i}'] = np.ascontiguousarray(
                wv.T[:, c * 128:(c + 1) * 128].reshape(ND, 128, 128)).astype(bf16)
            wo = out_proj_w[i]
            m[f'woT{i}'] = np.ascontiguousarray(
                wo.T[c * 128:(c + 1) * 128, :].reshape(128, ND, 128).transpose(1, 0, 2)).astype(bf16)
        m['wOutT'] = np.ascontiguousarray(
            w_out.T[:, c * 32:(c + 1) * 32].reshape(ND, 128, 32)).astype(bf16)

        sm = _Smalls()
        sm.add('id8', np.vstack([np.eye(8, dtype=f32), np.zeros((120, 8), f32)]))
        # SEL [16, 128] (rows 0:16): SEL[h, p] = 1 iff p//8 == h ;  MASK [128, 8]
        sel = np.zeros((128, 128), f32)
        for p in range(128):
            sel[p // 8, p] = 1.0
        sm.add('sel', sel)
        mask = np.zeros((128, 8), f32)
        for p in range(128):
            mask[p, p % 8] = 1.0
        sm.add('mask', mask)
        if flags['use_bq']:
            sm.add('bqT', _vec_T(in_proj_b[0, 0:D]))
        if flags['use_bv']:
            bv0 = in_proj_b[0, 2 * D:]
            bvexp = np.zeros((128, 64), f32)
            for p in range(128):
                bvexp[p, :] = bv0[(p // 8) * 64:(p // 8) * 64 + 64]
            sm.add('bvExp0', bvexp)
            for i in range(1, L):
                sm.add(f'bvT{i}', in_proj_b[i, 2 * D + c * 128: 2 * D + (c + 1) * 128].reshape(128, 1))
        if flags['use_bo']:
            sm.add('boT0s', out_proj_b[0, c * 128:(c + 1) * 128].reshape(128, 1))
            for i in range(1, L):
                sm.add(f'boT{i}', _vec_T(out_proj_b[i]))
        if flags['use_b1']:
            for i in range(L):
                sm.add(f'b1T{i}', lin1_b[i, c * 256:(c + 1) * 256].reshape(2, 128).T)
        if flags['use_b2']:
            for i in range(L):
                sm.add(f'b2T{i}', _vec_T(lin2_b[i]))
        if flags['use_ln1']:
            for i in range(L):
                sm.add(f'ln1wT{i}', _vec_T(ln1_w[i]))
                sm.add(f'ln1bT{i}', _vec_T(ln1_b[i]))
        if flags['use_ln2']:
            for i in range(L):
                sm.add(f'ln2wT{i}', _vec_T(ln2_w[i]))
                sm.add(f'ln2bT{i}', _vec_T(ln2_b[i]))
        if flags['use_bout']:
            bo = np.zeros((128, 1), f32)
            bo[0:32, 0] = b_out[c * 32:(c + 1) * 32]
            sm.add('boutT', bo)
        m['smalls'] = sm.pack()
        in_maps.append(m)

    meta = {'smalls': sm, 'n_smalls': in_maps[0]['smalls'].shape[1]}
    return in_maps, flags, meta


def _build(flags, n_smalls, smalls_obj):
    import concourse.bass as bass
    import concourse.tile as tile
    import concourse.mybir as mybir
    from concourse import bacc

    F32 = mybir.dt.float32
    BF16 = mybir.dt.bfloat16
    AF = mybir.ActivationFunctionType
    ALU = mybir.AluOpType

    nc = bacc.Bacc("TRN2", target_bir_lowering=False, num_devices=NCORE)

    def din(name, shape, dt=F32):
        return nc.dram_tensor(name, shape, dt, kind="ExternalInput")

    tgtC = din('tgtC', [B, NT, 128, D])
    peC = din('peC', [NT, 128, D])
    qT_in = din('qT', [128, ND, B])
    wqT_in = din('wqT', [ND, 128, D], BF16)
    wkn_in = din('wkn', [8, 128, D], BF16)
    wvT0_in = din('wvT0', [ND, 128, D], BF16)
    woT0_in = din('woT0', [H, DH, 128], BF16)
    w1T_in = [din(f'w1T{i}', [2, ND, 128, 128], BF16) for i in range(L)]
    w2T_in = [din(f'w2T{i}', [2, ND, 128, 128], BF16) for i in range(L)]
    wvT_in = {i: din(f'wvT{i}', [ND, 128, 128], BF16) for i in range(1, L)}
    woT_in = {i: din(f'woT{i}', [ND, 128, 128], BF16) for i in range(1, L)}
    wOutT_in = din('wOutT', [ND, 128, 32], BF16)
    smalls_in = din('smalls', [128, n_smalls])
    out_t = nc.dram_tensor('out', [32, B], F32, kind="ExternalOutput")

    RG = [list(range(NCORE))]

    with tile.TileContext(nc) as tc:
        with (
            tc.tile_pool(name="persist", bufs=1) as pp,
            tc.tile_pool(name="state", bufs=2) as stp,
            tc.tile_pool(name="dram", bufs=2, space="DRAM") as dram,
            tc.tile_pool(name="dram_sh", bufs=2, space="DRAM") as dram_sh,
        ):
            # ---------- persistent constants / weights ----------
            sm = pp.tile([128, n_smalls], F32)
            nc.sync.dma_start(sm[:], smalls_in[:])

            def smc(name):
                off, k, _ = smalls_obj.cols[name]
                return sm[:, off:off + k]

            ones128 = pp.tile([128, 1], F32)
            nc.vector.memset(ones128[:], 1.0)
            ones8w = pp.tile([8, 128], F32)
            nc.vector.memset(ones8w[:], 1.0)
            eps8 = pp.tile([8, 1], F32)
            nc.vector.memset(eps8[:], 1e-5)

            pe_sb = pp.tile([128, NT, D], F32)
            nc.sync.dma_start(pe_sb[:], peC[:].rearrange("st p d -> p st d"))
            qT_t = pp.tile([128, ND, B], F32)
            nc.sync.dma_start(qT_t[:], qT_in[:])

            wvT0_sb = pp.tile([128, ND, D], BF16)
            nc.sync.dma_start(wvT0_sb[:], wvT0_in[:].rearrange("dt p d -> p dt d"))
            woT0_sb = pp.tile([DH, H, 128], BF16)
            nc.sync.dma_start(woT0_sb[:], woT0_in[:].rearrange("h dh j -> dh h j"))
            w1T_sb = []
            w2T_sb = []
            for i in range(L):
                t1 = pp.tile([128, 2, ND, 128], BF16, name=f'w1Ts{i}')
                nc.sync.dma_start(t1[:], w1T_in[i][:].rearrange("fs dt p fl -> p fs dt fl"))
                w1T_sb.append(t1)
                t2 = pp.tile([128, 2, ND, 128], BF16, name=f'w2Ts{i}')
                nc.sync.dma_start(t2[:], w2T_in[i][:].rearrange("fs jt p jl -> p fs jt jl"))
                w2T_sb.append(t2)
            wvT_sb, woT_sb = {}, {}
            for i in range(1, L):
                tv = pp.tile([128, ND, 128], BF16, name=f'wvTs{i}')
                nc.sync.dma_start(tv[:], wvT_in[i][:].rearrange("dt p vl -> p dt vl"))
                wvT_sb[i] = tv
                to = pp.tile([128, ND, 128], BF16, name=f'woTs{i}')
                nc.sync.dma_start(to[:], woT_in[i][:].rearrange("jt p jl -> p jt jl"))
                woT_sb[i] = to
            wOutT_sb = pp.tile([128, ND, 32], BF16)
            nc.sync.dma_start(wOutT_sb[:], wOutT_in[:].rearrange("dt p vl -> p dt vl"))

            qkT_sb = pp.tile([128, ND, 128], BF16)       # [e, dt, h*8+b]
            ctxT_all = pp.tile([128, ND, 128], BF16)     # [e, dt, h*8+b]
            den_stack = pp.tile([16, B], F32)

            # ---------- prologue: QT, qk_stack, qkT ----------
            with (
                tc.tile_pool(name="prol", bufs=1) as prp,
                tc.tile_pool(name="prol2", bufs=2) as prp2,
                tc.tile_pool(name="ps_pro", bufs=2, space="PSUM") as psp,
            ):
                wqT_sb = prp.tile([128, ND, D], BF16)
                nc.sync.dma_start(wqT_sb[:], wqT_in[:].rearrange("dt p d -> p dt d"))
                wkn_sb = prp.tile([128, 8, D], BF16)
                nc.sync.dma_start(wkn_sb[:], wkn_in[:].rearrange("pr p d -> p pr d"))

                qT16 = prp.tile([128, ND, B], BF16)
                nc.vector.tensor_copy(qT16[:], qT_t[:])

                QT_sb = prp.tile([128, ND, B], F32)       # [(h,dh) local, mt, b]
                for mt in range(ND):
                    qt_ps = psp.tile([128, B], F32, tag="qtps")
                    for dt in range(ND):
                        nc.tensor.matmul(qt_ps[:], wqT_sb[:, dt, mt * 128:(mt + 1) * 128],
                                         qT16[:, dt, :], start=(dt == 0), stop=(dt == ND - 1))
                    if flags['use_bq']:
                        nc.vector.tensor_scalar(out=QT_sb[:, mt, :], in0=qt_ps[:],
                                                scalar1=smc('bqT')[:, mt:mt + 1], scalar2=None,
                                                op0=ALU.add)
                    else:
                        nc.vector.tensor_copy(QT_sb[:, mt, :], qt_ps[:])

                # qk_stack[(h*8+b), e] via 8 accumulating block-diag matmuls:
                # pair pr's lhsT is zero except its two heads' column blocks.
                qk_stack = prp.tile([128, D], BF16)       # [(h*8+b), e]
                lps = prp.tile([128, 8, 128], BF16)
                nc.vector.memset(lps[:], 0.0)
                for pr in range(8):
                    nc.vector.tensor_copy(lps[0:64, pr, (2 * pr) * 8:(2 * pr) * 8 + 8],
                                          QT_sb[0:64, pr, :])
                    nc.vector.tensor_copy(lps[64:128, pr, (2 * pr + 1) * 8:(2 * pr + 1) * 8 + 8],
                                          QT_sb[64:128, pr, :])
                qk_ps = psp.tile([128, D], F32, tag="qkps")
                for pr in range(8):
                    for hf in range(2):
                        nc.tensor.matmul(qk_ps[:, hf * 512:(hf + 1) * 512], lps[:, pr, :],
                                         wkn_sb[:, pr, hf * 512:(hf + 1) * 512],
                                         start=(pr == 0), stop=(pr == 7))
                nc.vector.tensor_copy(qk_stack[:], qk_ps[:])
                nc.sync.dma_start_transpose(qkT_sb[:], qk_stack[:])

            # ---------- attention (S-sharded), per batch ----------
            with (
                tc.tile_pool(name="attn", bufs=3) as ap_natf,
                tc.tile_pool(name="attn2", bufs=2) as ap2,
                tc.tile_pool(name="ps_sc", bufs=1, space="PSUM") as ps_sc,
                tc.tile_pool(name="ps_ctx", bufs=2, space="PSUM") as ps_ctx,
                tc.tile_pool(name="ps_u", bufs=1, space="PSUM") as ps_u,
            ):
                for b in range(B):
                    nat16 = ap2.tile([128, NT, D], BF16, tag="nat16")
                    outT_b = ap2.tile([128, NT, ND, 128], BF16, tag="outT")
                    for st in range(NT):
                        natf = ap_natf.tile([128, D], F32, tag="natf")
                        nc.sync.dma_start(natf[:], tgtC[b, st, :, :])
                        nc.vector.tensor_tensor(out=nat16[:, st, :], in0=natf[:],
                                                in1=pe_sb[:, st, :], op=ALU.add)
                        nc.sync.dma_start_transpose(outT_b[:, st, :, :], nat16[:, st, :])
                    qkTb = ap2.tile([128, ND, 16], BF16, tag="qkTb")
                    nc.vector.tensor_copy(qkTb[:], qkT_sb[:, :, b:b + 121:8])
                    sc_ps = ps_sc.tile([16, SC], F32, tag="sc")
                    for dt in range(ND):
                        nc.tensor.matmul(sc_ps[:], qkTb[:, dt, :], outT_b[:, :, dt, :],
                                         start=(dt == 0), stop=(dt == ND - 1))
                    a16 = ap2.tile([16, SC], BF16, tag="a16")
                    nc.scalar.activation(out=a16[:], in_=sc_ps[:], func=AF.Exp,
                                         scale=SCALE, accum_out=den_stack[:, b:b + 1])
                    aT_b = ap2.tile([128, NT, 16], BF16, tag="aT")
                    nc.sync.dma_start_transpose(aT_b[:], a16[:])
                    ctx_ps = ps_ctx.tile([16, D], F32, tag="ctx")
                    for st in range(NT):
                        for hf in range(2):
                            nc.tensor.matmul(ctx_ps[:, hf * 512:(hf + 1) * 512],
                                             aT_b[:, st, :], nat16[:, st, hf * 512:(hf + 1) * 512],
                                             start=(st == 0), stop=(st == NT - 1))
                    ctx16 = ap2.tile([16, D], BF16, tag="ctx16")
                    nc.scalar.activation(out=ctx16[:], in_=ctx_ps[:], func=AF.Copy)
                    ctmp = ap2.tile([128, ND, 16], BF16, tag="ctmp")
                    nc.sync.dma_start_transpose(ctmp[:], ctx16[:])
                    nc.vector.tensor_copy(ctxT_all[:, :, b:b + 121:8], ctmp[:])

                # ----- u = ctx @ wvT0 (all heads x all b), diag-extract, den relayout
                u_ps = ps_u.tile([128, D], F32, tag="u")
                for dt in range(ND):
                    for hf in range(2):
                        nc.tensor.matmul(u_ps[:, hf * 512:(hf + 1) * 512],
                                         ctxT_all[:, dt, :], wvT0_sb[:, dt, hf * 512:(hf + 1) * 512],
                                         start=(dt == 0), stop=(dt == ND - 1))
                u_f = stp.tile([128, D], F32, tag="uf")
                nc.vector.tensor_copy(u_f[:], u_ps[:])
                u_sb = stp.tile([128, 65], F32, tag="usb")
                for h in range(H):
                    nc.gpsimd.dma_start(u_sb[h * 8:(h + 1) * 8, 0:64],
                                        u_f[h * 8:(h + 1) * 8, h * 64:(h + 1) * 64])
                # den relayout [16, 8] -> [128, 1] (p = h*8+b) via SEL matmul + mask
                dsel_ps = ps_sc.tile([128, 8], F32, tag="dsel")
                nc.tensor.matmul(dsel_ps[:], smc('sel')[0:16, :], den_stack[:],
                                 start=True, stop=True)
                dmask = stp.tile([128, 8], F32, tag="dmask")
                nc.vector.tensor_mul(dmask[:], dsel_ps[:], smc('mask'))
                nc.vector.tensor_reduce(u_sb[:, 64:65], dmask[:],
                                        axis=mybir.AxisListType.X, op=ALU.add)

            # ---------- u-sync (AllGather + rank-sum) ----------
            cc1_in = dram.tile([128, 65], F32, tag="cc1i")
            cc1_out = dram_sh.tile([128 * NCORE, 65], F32, addr_space="Shared", tag="cc1o")
            nc.sync.dma_start(cc1_in[:], u_sb[:])
            nc.gpsimd.collective_compute("AllGather", mybir.AluOpType.bypass,
                                         replica_groups=RG,
                                         ins=[cc1_in.opt()], outs=[cc1_out.opt()])
            g1 = stp.tile([128, NCORE, 65], F32, tag="g1")
            nc.sync.dma_start(g1[:], cc1_out[:].rearrange("(r p) x -> p r x", r=NCORE))
            t4 = stp.tile([128, 4, 65], F32, tag="t4")
            nc.vector.tensor_add(t4[:], g1[:, 0:4, :], g1[:, 4:8, :])
            t2 = stp.tile([128, 2, 65], F32, tag="t2")
            nc.vector.tensor_add(t2[:], t4[:, 0:2, :], t4[:, 2:4, :])
            t1 = stp.tile([128, 65], F32, tag="t1")
            nc.vector.tensor_add(t1[:], t2[:, 0, :], t2[:, 1, :])
            rd = stp.tile([128, 1], F32, tag="rd")
            nc.vector.reciprocal(rd[:], t1[:, 64:65])
            o16 = stp.tile([128, 128], BF16, tag="o16")
            nc.vector.memset(o16[:, 64:128], 0.0)
            nc.vector.tensor_scalar_mul(o16[:, 0:64], t1[:, 0:64], rd[:])
            if flags['use_bv']:
                nc.vector.tensor_tensor(out=o16[:, 0:64], in0=o16[:, 0:64],
                                        in1=smc('bvExp0'), op=ALU.add)
            xo = stp.tile([128, 128], BF16, tag="xo")
            nc.sync.dma_start_transpose(xo[:], o16[:])

            # ---------- helpers for the sequential layer stack ----------
            with (
                tc.tile_pool(name="lay", bufs=2) as lp_,
                tc.tile_pool(name="ps_mm", bufs=2, space="PSUM") as ps_mm,
                tc.tile_pool(name="ps_ln", bufs=1, space="PSUM") as ps_ln,
            ):
                def sync_partial(src_sb, tagn):
                    """AllGather [128, jt, b] f32 partial sums; returns [128, ND, B] summed."""
                    pin = dram.tile([D, B], F32, tag="pin")
                    pout = dram_sh.tile([D * NCORE, B], F32, addr_space="Shared", tag="pout")
                    nc.sync.dma_start(pin[:].rearrange("(jt p) b -> p jt b", jt=ND), src_sb[:])
                    nc.gpsimd.collective_compute("AllGather", mybir.AluOpType.bypass,
                                                 replica_groups=RG,
                                                 ins=[pin.opt()], outs=[pout.opt()])
                    gg = lp_.tile([128, NCORE, ND, B], F32, tag="gg")
                    nc.sync.dma_start(gg[:], pout[:].rearrange("(r jt p) b -> p r jt b", r=NCORE, jt=ND))
                    s4 = lp_.tile([128, 4, ND, B], F32, tag="s4")
                    nc.vector.tensor_add(s4[:], gg[:, 0:4], gg[:, 4:8])
                    s2 = lp_.tile([128, 2, ND, B], F32, tag="s2")
                    nc.vector.tensor_add(s2[:], s4[:, 0:2], s4[:, 2:4])
                    s1 = lp_.tile([128, ND, B], F32, tag="s1")
                    nc.vector.tensor_add(s1[:], s2[:, 0], s2[:, 1])
                    return s1

                def sync_slices(src_sb):
                    """AllGather exact [128, b] f32 slices; returns [128, ND, B]."""
                    pin = dram.tile([128, B], F32, tag="pin2")
                    pout = dram_sh.tile([128 * NCORE, B], F32, addr_space="Shared", tag="pout2")
                    nc.sync.dma_start(pin[:], src_sb[:])
                    nc.gpsimd.collective_compute("AllGather", mybir.AluOpType.bypass,
                                                 replica_groups=RG,
                                                 ins=[pin.opt()], outs=[pout.opt()])
                    gg = lp_.tile([128, ND, B], F32, tag="gg2")
                    nc.sync.dma_start(gg[:], pout[:].rearrange("(jt p) b -> p jt b", jt=ND))
                    return gg

                def emit_ln(x_f, wcol, bcol, use_affine, nm):
                    """LayerNorm over feature dim in [d_part, dt, b] layout.
                    Returns (y_f fp32 [128, ND, B], y16 bf16)."""
                    st_ps = ps_ln.tile([8, 9], F32, tag="st")
                    for dt in range(ND):
                        nc.tensor.matmul(st_ps[:, 0:1], x_f[:, dt, :], ones128[:],
                                         start=(dt == 0), stop=(dt == ND - 1))
                        nc.tensor.matmul(st_ps[:, 1:9], x_f[:, dt, :], x_f[:, dt, :],
                                         start=(dt == 0), stop=(dt == ND - 1))
                    gsb = lp_.tile([8, 9], F32, tag="gsb")
                    nc.vector.tensor_copy(gsb[:], st_ps[:])
                    gm = lp_.tile([8, 8], F32, tag="gm")
                    nc.vector.tensor_mul(gm[:], gsb[:, 1:9], smc('id8')[0:8, :])
                    mv = lp_.tile([8, 4], F32, tag="mv")  # cols: ss, m, var, mr
                    nc.vector.tensor_reduce(mv[:, 0:1], gm[:], axis=mybir.AxisListType.X, op=ALU.add)
                    nc.vector.tensor_scalar_mul(mv[:, 1:2], gsb[:, 0:1], 1.0 / D)
                    nc.vector.tensor_scalar_mul(mv[:, 0:1], mv[:, 0:1], 1.0 / D)
                    m2 = lp_.tile([8, 1], F32, tag="m2")
                    nc.vector.tensor_mul(m2[:], mv[:, 1:2], mv[:, 1:2])
                    nc.vector.tensor_sub(mv[:, 2:3], mv[:, 0:1], m2[:])
                    sq = lp_.tile([8, 1], F32, tag="sq")
                    nc.scalar.activation(out=sq[:], in_=mv[:, 2:3], func=AF.Sqrt, bias=eps8[:])
                    rstd = lp_.tile([8, 1], F32, tag="rstd")
                    nc.vector.reciprocal(rstd[:], sq[:])
                    nc.vector.tensor_mul(mv[:, 3:4], mv[:, 1:2], rstd[:])
                    dg = lp_.tile([8, 16], F32, tag="dg")
                    nc.vector.tensor_scalar_mul(dg[:, 0:8], smc('id8')[0:8, :], rstd[:])
                    nc.vector.tensor_scalar_mul(dg[:, 8:16], smc('id8')[0:8, :], mv[:, 3:4])
                    bc_ps = ps_ln.tile([128, 16], F32, tag="bc")
                    nc.tensor.matmul(bc_ps[:], ones8w[:], dg[:], start=True, stop=True)
                    bc_sb = lp_.tile([128, 16], F32, tag="bcs")
                    nc.vector.tensor_copy(bc_sb[:], bc_ps[:])
                    y_f = lp_.tile([128, ND, B], F32, tag=f"yf{nm}")
                    for dt in range(ND):
                        nc.vector.tensor_tensor(out=y_f[:, dt, :], in0=x_f[:, dt, :],
                                                in1=bc_sb[:, 0:8], op=ALU.mult)
                        nc.gpsimd.tensor_tensor(out=y_f[:, dt, :], in0=y_f[:, dt, :],
                                                in1=bc_sb[:, 8:16], op=ALU.subtract)
                        if use_affine:
                            nc.vector.tensor_scalar(out=y_f[:, dt, :], in0=y_f[:, dt, :],
                                                    scalar1=wcol[:, dt:dt + 1],
                                                    scalar2=bcol[:, dt:dt + 1],
                                                    op0=ALU.mult, op1=ALU.add)
                    y16 = lp_.tile([128, ND, B], BF16, tag=f"y16{nm}")
                    nc.vector.tensor_copy(y16[:], y_f[:])
                    return y_f, y16

                def emit_ffn(i, y_f, y16, nm):
                    h_ps = ps_mm.tile([128, 2, B], F32, tag="mm")
                    for fs in range(2):
                        for dt in range(ND):
                            nc.tensor.matmul(h_ps[:, fs, :], w1T_sb[i][:, fs, dt, :],
                                             y16[:, dt, :], start=(dt == 0), stop=(dt == ND - 1))
                    h16 = lp_.tile([128, 2, B], BF16, tag="h16")
                    for fs in range(2):
                        if flags['use_b1']:
                            nc.scalar.activation(out=h16[:, fs, :], in_=h_ps[:, fs, :],
                                                 func=AF.Relu,
                                                 bias=smc(f'b1T{i}')[:, fs:fs + 1])
                        else:
                            nc.scalar.activation(out=h16[:, fs, :], in_=h_ps[:, fs, :],
                                                 func=AF.Relu)
                    ff_ps = ps_mm.tile([128, ND, B], F32, tag="mm")
                    for jt in range(ND):
                        for fs in range(2):
                            nc.tensor.matmul(ff_ps[:, jt, :], w2T_sb[i][:, fs, jt, :],
                                             h16[:, fs, :], start=(fs == 0), stop=(fs == 1))
                    ff_sb = lp_.tile([128, ND, B], F32, tag="ffsb")
                    nc.vector.tensor_copy(ff_sb[:], ff_ps[:])
                    s1 = sync_partial(ff_sb, f"ff{nm}")
                    x2 = lp_.tile([128, ND, B], F32, tag=f"x2{nm}")
                    nc.vector.tensor_add(x2[:], s1[:], y_f[:])
                    if flags['use_b2']:
                        for dt in range(ND):
                            nc.vector.tensor_scalar(out=x2[:, dt, :], in0=x2[:, dt, :],
                                                    scalar1=smc(f'b2T{i}')[:, dt:dt + 1],
                                                    scalar2=None, op0=ALU.add)
                    return emit_ln(x2, smc(f'ln2wT{i}') if flags['use_ln2'] else None,
                                   smc(f'ln2bT{i}') if flags['use_ln2'] else None,
                                   flags['use_ln2'], f"l2{nm}")

                # ---------- layer 0: out_proj from xo, then LN1, FFN, LN2 ----------
                z_ps = ps_mm.tile([128, B], F32, tag="mm")
                for h in range(H):
                    nc.tensor.matmul(z_ps[:], woT0_sb[:, h, :], xo[0:64, h * 8:(h + 1) * 8],
                                     start=(h == 0), stop=(h == H - 1))
                z_sb = lp_.tile([128, B], F32, tag="zsb0")
                if flags['use_bo']:
                    nc.vector.tensor_scalar(out=z_sb[:], in0=z_ps[:],
                                            scalar1=smc('boT0s'), scalar2=None, op0=ALU.add)
                else:
                    nc.vector.tensor_copy(z_sb[:], z_ps[:])
                zT = sync_slices(z_sb)
                x1 = lp_.tile([128, ND, B], F32, tag="x1l0")
                nc.vector.tensor_add(x1[:], zT[:], qT_t[:])
                y_f, y16 = emit_ln(x1, smc('ln1wT0') if flags['use_ln1'] else None,
                                   smc('ln1bT0') if flags['use_ln1'] else None,
                                   flags['use_ln1'], "l1L0")
                x_f, x16 = emit_ffn(0, y_f, y16, "L0")

                # ---------- layers 1..5 ----------
                for i in range(1, L):
                    v_ps = ps_mm.tile([128, B], F32, tag="mm")
                    for dt in range(ND):
                        nc.tensor.matmul(v_ps[:], wvT_sb[i][:, dt, :], x16[:, dt, :],
                                         start=(dt == 0), stop=(dt == ND - 1))
                    v16 = lp_.tile([128, B], BF16, tag="v16")
                    if flags['use_bv']:
                        nc.vector.tensor_scalar(out=v16[:], in0=v_ps[:],
                                                scalar1=smc(f'bvT{i}'), scalar2=None, op0=ALU.add)
                    else:
                        nc.vector.tensor_copy(v16[:], v_ps[:])
                    zl_ps = ps_mm.tile([128, ND, B], F32, tag="mm")
                    for jt in range(ND):
                        nc.tensor.matmul(zl_ps[:, jt, :], woT_sb[i][:, jt, :], v16[:],
                                         start=True, stop=True)
                    zl_sb = lp_.tile([128, ND, B], F32, tag="zlsb")
                    nc.vector.tensor_copy(zl_sb[:], zl_ps[:])
                    s1 = sync_partial(zl_sb, f"z{i}")
                    xa = lp_.tile([128, ND, B], F32, tag="xa")
                    nc.vector.tensor_add(xa[:], s1[:], x_f[:])
                    if flags['use_bo']:
                        for dt in range(ND):
                            nc.vector.tensor_scalar(out=xa[:, dt, :], in0=xa[:, dt, :],
                                                    scalar1=smc(f'boT{i}')[:, dt:dt + 1],
                                                    scalar2=None, op0=ALU.add)
                    y_f, y16 = emit_ln(xa, smc(f'ln1wT{i}') if flags['use_ln1'] else None,
                                       smc(f'ln1bT{i}') if flags['use_ln1'] else None,
                                       flags['use_ln1'], f"l1L{i}")
                    x_f, x16 = emit_ffn(i, y_f, y16, f"L{i}")

                # ---------- logits ----------
                lg_ps = ps_mm.tile([32, B], F32, tag="mm")
                for dt in range(ND):
                    nc.tensor.matmul(lg_ps[:], wOutT_sb[:, dt, :], x16[:, dt, :],
                                     start=(dt == 0), stop=(dt == ND - 1))
                lg_sb = lp_.tile([32, B], F32, tag="lgsb")
                if flags['use_bout']:
                    nc.vector.tensor_scalar(out=lg_sb[:], in0=lg_ps[:],
                                            scalar1=smc('boutT')[0:32, :], scalar2=None,
                                            op0=ALU.add)
                else:
                    nc.vector.tensor_copy(lg_sb[:], lg_ps[:])
                nc.sync.dma_start(out_t[:], lg_sb[:])

    nc.compile()
    return nc


def kernel(**inputs):
    import sys
    if '/opt/trn_rl_repo' not in sys.path:
        sys.path.insert(0, '/opt/trn_rl_repo')
    from concourse.bass_utils import run_bass_kernel_spmd

    in_maps, flags, meta = _prep_inputs(inputs)
    key = tuple(sorted(flags.items())) + (meta['n_smalls'],)
    if key not in _BUILT:
        _BUILT[key] = _build(flags, meta['n_smalls'], meta['smalls'])
    nc = _BUILT[key]

    res = run_bass_kernel_spmd(nc, in_maps, core_ids=list(range(NCORE)))
    logits = np.zeros((B, 1, V), np.float32)
    for c in range(NCORE):
        o = res.results[c]['out']            # [32, B]
        logits[:, 0, c * 32:(c + 1) * 32] = o.T
    return logits



# revision 7
# speedup vs baseline: 14461.3662x; 14461.3662x over previous
"""Trainium2 Bass kernel for nn_CachedDecoderOnly (cached decode step).

Strategy (8 NeuronCores, SPMD — same NEFF, per-core data):
  - Host precomputes nat = tgt + pe (bf16) in BOTH layouts (seq-major and
    feature-major) and qk = (q@wq^T+bq) @ wk per head (fp32), so the device
    does no PE-add, no big transposes, and no q/k projections.
  - Layer-0 attention is algebraically folded: scores = qk @ nat^T,
    ctx = softmax-unnormalized @ nat, u = ctx @ wv0^T; the sequence
    (S=4096) is sharded over the 8 cores; partial (numerator, denominator)
    pairs are combined with one AllGather + on-device rank-sum.
  - Layers 1..5 collapse (softmax over one key) to x -> wv -> wo -> LN ->
    FFN -> LN, run tensor-parallel over the 8 cores in a feature-major
    [d_partition, batch_free] layout with one AllGather + rank-sum per
    matmul pair. All collective staging is p-major so every DMA moves
    contiguous 256B+ runs per partition.
  - Weights stream as bf16 (fp32 accumulation in PSUM); weight DMAs are
    issued after the attention loads so attention starts immediately.
"""

import numpy as np

B, S, D, H, DH, FF, V, L = 8, 4096, 1024, 16, 64, 2048, 256, 6
NCORE = 8
SC = S // NCORE          # 512 sequence rows per core per batch
NT = SC // 128           # 4 s-tiles
ND = D // 128            # 8 d-tiles
SCALE = 1.0 / 8.0        # 1/sqrt(DH)

_BUILT = {}


def _pe_np():
    pos = np.arange(S, dtype=np.float32)[:, None]
    div = np.exp(np.arange(0, D, 2, dtype=np.float32) * (-np.log(10000.0) / D))
    pe = np.zeros((S, D), np.float32)
    pe[:, 0::2] = np.sin(pos * div)
    pe[:, 1::2] = np.cos(pos * div)
    return pe


class _Smalls:
    """Packs per-core [128, n] fp32 constant columns; returns column slices."""

    def __init__(self):
        self.cols = {}
        self.n = 0

    def add(self, name, arr):  # arr [128, k]
        arr = np.asarray(arr, np.float32)
        assert arr.shape[0] == 128
        k = arr.shape[1] if arr.ndim == 2 else 1
        arr = arr.reshape(128, k)
        self.cols[name] = (self.n, k, arr)
        self.n += k

    def pack(self):
        out = np.zeros((128, max(self.n, 1)), np.float32)
        for off, k, arr in self.cols.values():
            out[:, off:off + k] = arr
        return out


def _vec_T(v):
    """[D] fp32 -> [128, ND] feature-major tile layout: out[p, dt] = v[dt*128+p]."""
    return np.ascontiguousarray(v.reshape(ND, 128).T)


def _prep_inputs(inputs):
    """Host-side weight re-layout + sharding. Returns (in_maps, flags, meta)."""
    f32 = np.float32
    g = {k: np.asarray(v) for k, v in inputs.items()}
    pe = _pe_np()

    in_proj_w = g['in_proj_w'].astype(f32)
    in_proj_b = g['in_proj_b'].astype(f32)
    out_proj_w = g['out_proj_w'].astype(f32)
    out_proj_b = g['out_proj_b'].astype(f32)
    lin1_w, lin1_b = g['lin1_w'].astype(f32), g['lin1_b'].astype(f32)
    lin2_w, lin2_b = g['lin2_w'].astype(f32), g['lin2_b'].astype(f32)
    ln1_w, ln1_b = g['ln1_w'].astype(f32), g['ln1_b'].astype(f32)
    ln2_w, ln2_b = g['ln2_w'].astype(f32), g['ln2_b'].astype(f32)
    w_out, b_out = g['w_out'].astype(f32), g['b_out'].astype(f32)
    tgt = g['tgt'].astype(f32)

    flags = {
        'use_bv': bool(np.any(in_proj_b[:, 2 * D:] != 0)),
        'use_bo': bool(np.any(out_proj_b != 0)),
        'use_b1': bool(np.any(lin1_b != 0)),
        'use_b2': bool(np.any(lin2_b != 0)),
        'use_bout': bool(np.any(b_out != 0)),
        'use_ln1': bool(np.any(ln1_w != 1) or np.any(ln1_b != 0)),
        'use_ln2': bool(np.any(ln2_w != 1) or np.any(ln2_b != 0)),
    }

    import ml_dtypes
    bf16 = ml_dtypes.bfloat16

    # q / Q / qk on host (fp32, exact): scores = qk . nat  (bk drops out of
    # softmax as a per-(b,h) constant; bq folded here)
    q = tgt[:, -1, :] + pe[-1]                                   # [B, D]
    Q = q @ in_proj_w[0, 0:D].T + in_proj_b[0, 0:D]              # [B, D]
    wk = in_proj_w[0, D:2 * D]                                   # [D, D]
    qk = np.zeros((B, H, D), f32)
    for h in range(H):
        qk[:, h, :] = Q[:, h * DH:(h + 1) * DH] @ wk[h * DH:(h + 1) * DH, :]
    # qkT [128, ND, B, 16]: qkT[p, dt, b, h] = qk[b, h, dt*128+p]
    qkT = np.ascontiguousarray(qk.transpose(2, 0, 1).reshape(ND, 128, B, H)
                               .transpose(1, 0, 2, 3)).astype(bf16)
    qT_t = np.ascontiguousarray(q.T.reshape(ND, 128, B).transpose(1, 0, 2))  # [128, ND, B] f32

    wv0 = in_proj_w[0, 2 * D:]                                   # [D, D]
    # wvT0 [128, ND, D]: wvT0[p, et, n] = wv0[n, et*128+p]
    wvT0 = np.ascontiguousarray(wv0.T.reshape(ND, 128, D).transpose(1, 0, 2)).astype(bf16)

    shared = {'qkT': qkT, 'qT': qT_t, 'wvT0': wvT0}

    in_maps = []
    for c in range(NCORE):
        m = dict(shared)
        natc = (tgt[:, c * SC:(c + 1) * SC, :] + pe[c * SC:(c + 1) * SC]).astype(bf16)
        # natC [B, 128, NT, D]: natC[b, p, st, d] = nat[b, st*128+p, d]
        m['natC'] = np.ascontiguousarray(
            natc.reshape(B, NT, 128, D).transpose(0, 2, 1, 3))
        # natT [B, 128, ND, SC]: natT[b, p, dt, s] = nat[b, s, dt*128+p]
        m['natT'] = np.ascontiguousarray(
            natc.transpose(0, 2, 1).reshape(B, ND, 128, SC).transpose(0, 2, 1, 3))
        # woT0 [128, ND, 128]: woT0[e_l, dt, p] = wo0[dt*128+p, c*128+e_l]
        m['woT0'] = np.ascontiguousarray(
            out_proj_w[0].T[c * 128:(c + 1) * 128].reshape(128, ND, 128)).astype(bf16)
        for i in range(L):
            w1T = lin1_w[i].T[:, c * 256:(c + 1) * 256]          # [1024, 256]
            m[f'w1T{i}'] = np.ascontiguousarray(
                w1T.reshape(ND, 128, 2, 128).transpose(1, 2, 0, 3)).astype(bf16)
            w2T = lin2_w[i].T[c * 256:(c + 1) * 256, :]          # [256, 1024]
            m[f'w2T{i}'] = np.ascontiguousarray(
                w2T.reshape(2, 128, ND, 128).transpose(1, 0, 2, 3)).astype(bf16)
        for i in range(1, L):
            wv = in_proj_w[i, 2 * D:]
            # wvT [128, ND, 128]: wvT[p, dt, jl] = wv[c*128+jl, dt*128+p]
            m[f'wvT{i}'] = np.ascontiguousarray(
                wv.T[:, c * 128:(c + 1) * 128].reshape(ND, 128, 128).transpose(1, 0, 2)).astype(bf16)
            wo = out_proj_w[i]
            # woT [128, ND, 128]: woT[jp, dt, p] = wo[dt*128+p, c*128+jp]
            m[f'woT{i}'] = np.ascontiguousarray(
                wo.T[c * 128:(c + 1) * 128, :].reshape(128, ND, 128)).astype(bf16)
        # wOutT [128, ND, 32]: wOutT[p, dt, vl] = w_out[c*32+vl, dt*128+p]
        m['wOutT'] = np.ascontiguousarray(
            w_out.T[:, c * 32:(c + 1) * 32].reshape(ND, 128, 32).transpose(1, 0, 2)).astype(bf16)

        sm = _Smalls()
        sm.add('id8', np.vstack([np.eye(8, dtype=f32), np.zeros((120, 8), f32)]))
        # den relayout (hb = b*16+h): sel[h, p] = 1 iff p%16 == h ; mask[p, b] = 1 iff p//16 == b
        sel = np.zeros((128, 128), f32)
        for p in range(128):
            sel[p % 16, p] = 1.0
        sm.add('sel', sel)
        mask = np.zeros((128, 8), f32)
        for p in range(128):
            mask[p, p // 16] = 1.0
        sm.add('mask', mask)
        # per-core head-pair select: osel[hb, m] = 1 iff hb == (m%8)*16 + 2c + m//8
        osel = np.zeros((128, 16), f32)
        for mm_ in range(16):
            osel[(mm_ % 8) * 16 + 2 * c + mm_ // 8, mm_] = 1.0
        sm.add('osel', osel)
        if flags['use_bv']:
            bv0 = in_proj_b[0, 2 * D:]
            bvexp = np.zeros((128, 64), f32)
            for p in range(128):
                bvexp[p, :] = bv0[(p % 16) * 64:(p % 16) * 64 + 64]
            sm.add('bvExp0', bvexp)
            for i in range(1, L):
                sm.add(f'bvT{i}', in_proj_b[i, 2 * D + c * 128: 2 * D + (c + 1) * 128].reshape(128, 1))
        if flags['use_bo']:
            for i in range(L):
                sm.add(f'boT{i}', _vec_T(out_proj_b[i]))
        if flags['use_b1']:
            for i in range(L):
                sm.add(f'b1T{i}', lin1_b[i, c * 256:(c + 1) * 256].reshape(2, 128).T)
        if flags['use_b2']:
            for i in range(L):
                sm.add(f'b2T{i}', _vec_T(lin2_b[i]))
        if flags['use_ln1']:
            for i in range(L):
                sm.add(f'ln1wT{i}', _vec_T(ln1_w[i]))
                sm.add(f'ln1bT{i}', _vec_T(ln1_b[i]))
        if flags['use_ln2']:
            for i in range(L):
                sm.add(f'ln2wT{i}', _vec_T(ln2_w[i]))
                sm.add(f'ln2bT{i}', _vec_T(ln2_b[i]))
        if flags['use_bout']:
            bo = np.zeros((128, 1), f32)
            bo[0:32, 0] = b_out[c * 32:(c + 1) * 32]
            sm.add('boutT', bo)
        m['smalls'] = sm.pack()
        in_maps.append(m)

    meta = {'smalls': sm, 'n_smalls': in_maps[0]['smalls'].shape[1]}
    return in_maps, flags, meta


def _build(flags, n_smalls, smalls_obj):
    import concourse.bass as bass
    import concourse.tile as tile
    import concourse.mybir as mybir
    from concourse import bacc

    F32 = mybir.dt.float32
    BF16 = mybir.dt.bfloat16
    AF = mybir.ActivationFunctionType
    ALU = mybir.AluOpType

    nc = bacc.Bacc("TRN2", target_bir_lowering=False, num_devices=NCORE)

    def din(name, shape, dt=F32):
        return nc.dram_tensor(name, shape, dt, kind="ExternalInput")

    natC_in = din('natC', [B, 128, NT, D], BF16)
    natT_in = din('natT', [B, 128, ND, SC], BF16)
    qkT_in = din('qkT', [128, ND, B, 16], BF16)
    qT_in = din('qT', [128, ND, B])
    wvT0_in = din('wvT0', [128, ND, D], BF16)
    woT0_in = din('woT0', [128, ND, 128], BF16)
    w1T_in = [din(f'w1T{i}', [128, 2, ND, 128], BF16) for i in range(L)]
    w2T_in = [din(f'w2T{i}', [128, 2, ND, 128], BF16) for i in range(L)]
    wvT_in = {i: din(f'wvT{i}', [128, ND, 128], BF16) for i in range(1, L)}
    woT_in = {i: din(f'woT{i}', [128, ND, 128], BF16) for i in range(1, L)}
    wOutT_in = din('wOutT', [128, ND, 32], BF16)
    smalls_in = din('smalls', [128, n_smalls])
    out_t = nc.dram_tensor('out', [32, B], F32, kind="ExternalOutput")

    RG = [list(range(NCORE))]

    with tile.TileContext(nc) as tc:
        with (
            tc.tile_pool(name="persist", bufs=1) as pp,
            tc.tile_pool(name="state", bufs=2) as stp,
            tc.tile_pool(name="dram", bufs=2, space="DRAM") as dram,
            tc.tile_pool(name="dram_sh", bufs=2, space="DRAM") as dram_sh,
        ):
            sm = pp.tile([128, n_smalls], F32)
            nc.sync.dma_start(sm[:], smalls_in[:])

            def smc(name):
                off, k, _ = smalls_obj.cols[name]
                return sm[:, off:off + k]

            ones128 = pp.tile([128, 1], F32)
            nc.vector.memset(ones128[:], 1.0)
            ones8w = pp.tile([8, 128], F32)
            nc.vector.memset(ones8w[:], 1.0)
            eps8 = pp.tile([8, 1], F32)
            nc.vector.memset(eps8[:], 1e-5)

            qkT_sb = pp.tile([128, ND, B, 16], BF16)
            nc.sync.dma_start(qkT_sb[:], qkT_in[:])
            qT_t = pp.tile([128, ND, B], F32)
            nc.sync.dma_start(qT_t[:], qT_in[:])

            den_stack = pp.tile([16, B], F32)
            ctxT = pp.tile([128, ND, 128], BF16)     # [e_local, et, hb], hb = b*16+h

            # ---------- attention (S-sharded), per batch ----------
            with (
                tc.tile_pool(name="attn", bufs=3) as ap_,
                tc.tile_pool(name="ps_sc", bufs=2, space="PSUM") as ps_sc,
                tc.tile_pool(name="ps_ctx", bufs=2, space="PSUM") as ps_ctx,
            ):
                with nc.named_scope("attn"):
                    for b in range(B):
                        natT_b = ap_.tile([128, ND, SC], BF16, tag="natT")
                        nc.sync.dma_start(natT_b[:], natT_in[b])
                        natC_b = ap_.tile([128, NT, D], BF16, tag="natC")
                        nc.sync.dma_start(natC_b[:], natC_in[b])
                        sc_ps = ps_sc.tile([16, SC], F32, tag="sc")
                        for dt in range(ND):
                            nc.tensor.matmul(sc_ps[:], qkT_sb[:, dt, b, :], natT_b[:, dt, :],
                                             start=(dt == 0), stop=(dt == ND - 1))
                        a16 = ap_.tile([16, SC], BF16, tag="a16")
                        nc.scalar.activation(out=a16[:], in_=sc_ps[:], func=AF.Exp,
                                             scale=SCALE, accum_out=den_stack[:, b:b + 1])
                        aT = ap_.tile([128, NT, 16], BF16, tag="aT")
                        nc.sync.dma_start_transpose(aT[:], a16[:])
                        ctx_ps = ps_ctx.tile([16, D], F32, tag="ctx")
                        for st in range(NT):
                            for hf in range(2):
                                nc.tensor.matmul(ctx_ps[:, hf * 512:(hf + 1) * 512],
                                                 aT[:, st, :], natC_b[:, st, hf * 512:(hf + 1) * 512],
                                                 start=(st == 0), stop=(st == NT - 1))
                        ctx16 = ap_.tile([16, D], BF16, tag="ctx16")
                        nc.vector.tensor_copy(ctx16[:], ctx_ps[:])
                        nc.sync.dma_start_transpose(ctxT[:, :, b * 16:(b + 1) * 16], ctx16[:])

            # ---------- weight preloads (issued after attention loads) ----------
            wvT0_sb = pp.tile([128, ND, D], BF16)
            nc.sync.dma_start(wvT0_sb[:], wvT0_in[:])
            woT0_sb = pp.tile([128, ND, 128], BF16)
            nc.sync.dma_start(woT0_sb[:], woT0_in[:])
            w1T_sb, w2T_sb = [], []
            wvT_sb, woT_sb = {}, {}
            for i in range(L):
                t1 = pp.tile([128, 2, ND, 128], BF16, name=f'w1Ts{i}')
                nc.sync.dma_start(t1[:], w1T_in[i][:])
                w1T_sb.append(t1)
                t2 = pp.tile([128, 2, ND, 128], BF16, name=f'w2Ts{i}')
                nc.sync.dma_start(t2[:], w2T_in[i][:])
                w2T_sb.append(t2)
                if i >= 1:
                    tv = pp.tile([128, ND, 128], BF16, name=f'wvTs{i}')
                    nc.sync.dma_start(tv[:], wvT_in[i][:])
                    wvT_sb[i] = tv
                    to = pp.tile([128, ND, 128], BF16, name=f'woTs{i}')
                    nc.sync.dma_start(to[:], woT_in[i][:])
                    woT_sb[i] = to
            wOutT_sb = pp.tile([128, ND, 32], BF16)
            nc.sync.dma_start(wOutT_sb[:], wOutT_in[:])

            # ---------- u = ctx @ wv0^T (all heads x batches), diag, den ----------
            with (
                tc.tile_pool(name="upool", bufs=1) as up_,
                tc.tile_pool(name="ps_u", bufs=1, space="PSUM") as ps_u,
                tc.tile_pool(name="ps_us", bufs=1, space="PSUM") as ps_us,
            ):
                with nc.named_scope("u"):
                    u_ps = ps_u.tile([128, D], F32, tag="u")
                    for et in range(ND):
                        for hf in range(2):
                            nc.tensor.matmul(u_ps[:, hf * 512:(hf + 1) * 512],
                                             ctxT[:, et, :], wvT0_sb[:, et, hf * 512:(hf + 1) * 512],
                                             start=(et == 0), stop=(et == ND - 1))
                    u_f = up_.tile([128, D], F32)
                    nc.vector.tensor_copy(u_f[:], u_ps[:])
                    u_sb = up_.tile([128, 65], F32)
                    for h in range(H):
                        nc.gpsimd.dma_start(u_sb[h:h + 113:16, 0:64],
                                            u_f[h:h + 113:16, h * 64:(h + 1) * 64])
                    dsel_ps = ps_us.tile([128, 8], F32, tag="dsel")
                    nc.tensor.matmul(dsel_ps[:], smc('sel')[0:16, :], den_stack[:],
                                     start=True, stop=True)
                    dmask = up_.tile([128, 8], F32)
                    nc.vector.tensor_mul(dmask[:], dsel_ps[:], smc('mask'))
                    nc.vector.tensor_reduce(u_sb[:, 64:65], dmask[:],
                                            axis=mybir.AxisListType.X, op=ALU.add)

                # ---------- u-sync (AllGather + rank-sum) ----------
                with nc.named_scope("usync"):
                    cc1_in = dram.tile([128, 65], F32, tag="cc1i")
                    cc1_out = dram_sh.tile([128 * NCORE, 65], F32, addr_space="Shared", tag="cc1o")
                    nc.sync.dma_start(cc1_in[:], u_sb[:])
                    nc.gpsimd.collective_compute("AllGather", mybir.AluOpType.bypass,
                                                 replica_groups=RG,
                                                 ins=[cc1_in.opt()], outs=[cc1_out.opt()])
                    g1 = stp.tile([128, NCORE, 65], F32, tag="g1")
                    nc.sync.dma_start(g1[:], cc1_out[:].rearrange("(r p) x -> p r x", r=NCORE))
                    t4 = stp.tile([128, 4, 65], F32, tag="t4")
                    nc.vector.tensor_add(t4[:], g1[:, 0:4, :], g1[:, 4:8, :])
                    t2 = stp.tile([128, 2, 65], F32, tag="t2")
                    nc.vector.tensor_add(t2[:], t4[:, 0:2, :], t4[:, 2:4, :])
                    t1 = stp.tile([128, 65], F32, tag="t1")
                    nc.vector.tensor_add(t1[:], t2[:, 0, :], t2[:, 1, :])
                    rd = stp.tile([128, 1], F32, tag="rd")
                    nc.vector.reciprocal(rd[:], t1[:, 64:65])
                    o16 = stp.tile([128, 64], F32, tag="o16")
                    nc.vector.tensor_scalar_mul(o16[:], t1[:, 0:64], rd[:])
                    if flags['use_bv']:
                        nc.vector.tensor_tensor(out=o16[:], in0=o16[:],
                                                in1=smc('bvExp0'), op=ALU.add)
                    # per-core head-pair select -> u_c [128, B]
                    os_ps = ps_us.tile([16, 64], F32, tag="osps")
                    nc.tensor.matmul(os_ps[:], smc('osel'), o16[:], start=True, stop=True)
                    os16 = stp.tile([16, 128], BF16, tag="os16")
                    nc.vector.memset(os16[:, 64:128], 0.0)
                    nc.vector.tensor_copy(os16[:, 0:64], os_ps[:])
                    xoc = stp.tile([128, 16], BF16, tag="xoc")
                    nc.sync.dma_start_transpose(xoc[:], os16[:])
                    u_c = stp.tile([128, B], BF16, tag="uc")
                    nc.gpsimd.dma_start(u_c[0:64, :], xoc[0:64, 0:8])
                    nc.gpsimd.dma_start(u_c[64:128, :], xoc[0:64, 8:16])

            # ---------- sequential layer stack ----------
            with (
                tc.tile_pool(name="lay", bufs=2) as lp_,
                tc.tile_pool(name="ps_mm", bufs=2, space="PSUM") as ps_mm,
                tc.tile_pool(name="ps_ln", bufs=1, space="PSUM") as ps_ln,
            ):
                def bcol(col):
                    """[128, ND] per-dt column tile -> broadcast AP [128, ND, B]."""
                    return col.rearrange("p (nd a) -> p nd a", a=1).to_broadcast((128, ND, B))

                def sync_partial(src_sb, nm):
                    """AllGather p-major [128, ND*B] f32 partials; returns summed [128, ND, B]."""
                    pin = dram.tile([128, ND * B], F32, tag="pin")
                    pout = dram_sh.tile([128 * NCORE, ND * B], F32, addr_space="Shared", tag="pout")
                    nc.sync.dma_start(pin[:], src_sb[:].rearrange("p nd b -> p (nd b)"))
                    nc.gpsimd.collective_compute("AllGather", mybir.AluOpType.bypass,
                                                 replica_groups=RG,
                                                 ins=[pin.opt()], outs=[pout.opt()])
                    gg = lp_.tile([128, NCORE, ND, B], F32, tag="gg")
                    nc.sync.dma_start(gg[:], pout[:].rearrange("(r p) (nd b) -> p r nd b",
                                                               r=NCORE, nd=ND))
                    s4 = lp_.tile([128, 4, ND, B], F32, tag="s4")
                    nc.vector.tensor_add(s4[:], gg[:, 0:4], gg[:, 4:8])
                    s2 = lp_.tile([128, 2, ND, B], F32, tag="s2")
                    nc.vector.tensor_add(s2[:], s4[:, 0:2], s4[:, 2:4])
                    s1 = lp_.tile([128, ND, B], F32, tag="s1")
                    nc.vector.tensor_add(s1[:], s2[:, 0], s2[:, 1])
                    return s1

                def emit_ln(x_f, wname, bname, use_affine, nm):
                    """LayerNorm over features in [d_part, dt, b] layout -> (y f32, y bf16)."""
                    st_ps = ps_ln.tile([8, 9], F32, tag="st")
                    for dt in range(ND):
                        nc.tensor.matmul(st_ps[:, 0:1], x_f[:, dt, :], ones128[:],
                                         start=(dt == 0), stop=(dt == ND - 1))
                        nc.tensor.matmul(st_ps[:, 1:9], x_f[:, dt, :], x_f[:, dt, :],
                                         start=(dt == 0), stop=(dt == ND - 1))
                    gsb = lp_.tile([8, 9], F32, tag="gsb")
                    nc.vector.tensor_copy(gsb[:], st_ps[:])
                    gm = lp_.tile([8, 8], F32, tag="gm")
                    nc.vector.tensor_mul(gm[:], gsb[:, 1:9], smc('id8')[0:8, :])
                    mv = lp_.tile([8, 4], F32, tag="mv")  # cols: ss, m, var, mr
                    nc.vector.tensor_reduce(mv[:, 0:1], gm[:], axis=mybir.AxisListType.X, op=ALU.add)
                    nc.vector.tensor_scalar_mul(mv[:, 1:2], gsb[:, 0:1], 1.0 / D)
                    nc.vector.tensor_scalar_mul(mv[:, 0:1], mv[:, 0:1], 1.0 / D)
                    m2 = lp_.tile([8, 1], F32, tag="m2")
                    nc.vector.tensor_mul(m2[:], mv[:, 1:2], mv[:, 1:2])
                    nc.vector.tensor_sub(mv[:, 2:3], mv[:, 0:1], m2[:])
                    sq = lp_.tile([8, 1], F32, tag="sq")
                    nc.scalar.activation(out=sq[:], in_=mv[:, 2:3], func=AF.Sqrt, bias=eps8[:])
                    rstd = lp_.tile([8, 1], F32, tag="rstd")
                    nc.vector.reciprocal(rstd[:], sq[:])
                    nc.vector.tensor_mul(mv[:, 3:4], mv[:, 1:2], rstd[:])
                    dg = lp_.tile([8, 16], F32, tag="dg")
                    nc.vector.tensor_scalar_mul(dg[:, 0:8], smc('id8')[0:8, :], rstd[:])
                    nc.vector.tensor_scalar_mul(dg[:, 8:16], smc('id8')[0:8, :], mv[:, 3:4])
                    bc_ps = ps_ln.tile([128, 16], F32, tag="bc")
                    nc.tensor.matmul(bc_ps[:], ones8w[:], dg[:], start=True, stop=True)
                    bc_sb = lp_.tile([128, 16], F32, tag="bcs")
                    nc.vector.tensor_copy(bc_sb[:], bc_ps[:])
                    y_f = lp_.tile([128, ND, B], F32, tag=f"yf{nm}")
                    bc0 = bc_sb[:, 0:8].rearrange("p (a b) -> p a b", a=1).to_broadcast((128, ND, B))
                    bc1 = bc_sb[:, 8:16].rearrange("p (a b) -> p a b", a=1).to_broadcast((128, ND, B))
                    nc.vector.tensor_tensor(out=y_f[:], in0=x_f[:], in1=bc0, op=ALU.mult)
                    nc.vector.tensor_tensor(out=y_f[:], in0=y_f[:], in1=bc1, op=ALU.subtract)
                    if use_affine:
                        nc.vector.tensor_tensor(out=y_f[:], in0=y_f[:], in1=bcol(smc(wname)),
                                                op=ALU.mult)
                        nc.vector.tensor_tensor(out=y_f[:], in0=y_f[:], in1=bcol(smc(bname)),
                                                op=ALU.add)
                    y16 = lp_.tile([128, ND, B], BF16, tag=f"y16{nm}")
                    nc.vector.tensor_copy(y16[:], y_f[:])
                    return y_f, y16

                def emit_ffn(i, y_f, y16, nm):
                    h_ps = ps_mm.tile([128, 2, B], F32, tag="mm")
                    for fs in range(2):
                        for dt in range(ND):
                            nc.tensor.matmul(h_ps[:, fs, :], w1T_sb[i][:, fs, dt, :],
                                             y16[:, dt, :], start=(dt == 0), stop=(dt == ND - 1))
                    h16 = lp_.tile([128, 2, B], BF16, tag="h16")
                    for fs in range(2):
                        if flags['use_b1']:
                            nc.scalar.activation(out=h16[:, fs, :], in_=h_ps[:, fs, :],
                                                 func=AF.Relu,
                                                 bias=smc(f'b1T{i}')[:, fs:fs + 1])
                        else:
                            nc.scalar.activation(out=h16[:, fs, :], in_=h_ps[:, fs, :],
                                                 func=AF.Relu)
                    ff_ps = ps_mm.tile([128, ND, B], F32, tag="mm")
                    for jt in range(ND):
                        for fs in range(2):
                            nc.tensor.matmul(ff_ps[:, jt, :], w2T_sb[i][:, fs, jt, :],
                                             h16[:, fs, :], start=(fs == 0), stop=(fs == 1))
                    ff_sb = lp_.tile([128, ND, B], F32, tag="ffsb")
                    nc.vector.tensor_copy(ff_sb[:], ff_ps[:])
                    s1 = sync_partial(ff_sb, f"ff{nm}")
                    x2 = lp_.tile([128, ND, B], F32, tag=f"x2{nm}")
                    nc.vector.tensor_add(x2[:], s1[:], y_f[:])
                    if flags['use_b2']:
                        nc.vector.tensor_tensor(out=x2[:], in0=x2[:], in1=bcol(smc(f'b2T{i}')),
                                                op=ALU.add)
                    return emit_ln(x2, f'ln2wT{i}', f'ln2bT{i}', flags['use_ln2'], f"l2{nm}")

                # ---------- layer 0: out_proj partial (e-sharded), LN1, FFN, LN2 ----------
                with nc.named_scope("stack"):
                    z_ps = ps_mm.tile([128, ND, B], F32, tag="mm")
                    for dt in range(ND):
                        nc.tensor.matmul(z_ps[:, dt, :], woT0_sb[:, dt, :], u_c[:],
                                         start=True, stop=True)
                    z_sb = lp_.tile([128, ND, B], F32, tag="zsb0")
                    nc.vector.tensor_copy(z_sb[:], z_ps[:])
                    s1 = sync_partial(z_sb, "z0")
                    x1 = lp_.tile([128, ND, B], F32, tag="x1l0")
                    nc.vector.tensor_add(x1[:], s1[:], qT_t[:])
                    if flags['use_bo']:
                        nc.vector.tensor_tensor(out=x1[:], in0=x1[:], in1=bcol(smc('boT0')),
                                                op=ALU.add)
                    y_f, y16 = emit_ln(x1, 'ln1wT0', 'ln1bT0', flags['use_ln1'], "l1L0")
                    x_f, x16 = emit_ffn(0, y_f, y16, "L0")

                    # ---------- layers 1..5 ----------
                    for i in range(1, L):
                        v_ps = ps_mm.tile([128, B], F32, tag="mm")
                        for dt in range(ND):
                            nc.tensor.matmul(v_ps[:], wvT_sb[i][:, dt, :], x16[:, dt, :],
                                             start=(dt == 0), stop=(dt == ND - 1))
                        v16 = lp_.tile([128, B], BF16, tag="v16")
                        if flags['use_bv']:
                            nc.vector.tensor_scalar(out=v16[:], in0=v_ps[:],
                                                    scalar1=smc(f'bvT{i}'), scalar2=None,
                                                    op0=ALU.add)
                        else:
                            nc.vector.tensor_copy(v16[:], v_ps[:])
                        zl_ps = ps_mm.tile([128, ND, B], F32, tag="mm")
                        for dt in range(ND):
                            nc.tensor.matmul(zl_ps[:, dt, :], woT_sb[i][:, dt, :], v16[:],
                                             start=True, stop=True)
                        zl_sb = lp_.tile([128, ND, B], F32, tag="zlsb")
                        nc.vector.tensor_copy(zl_sb[:], zl_ps[:])
                        s1 = sync_partial(zl_sb, f"z{i}")
                        xa = lp_.tile([128, ND, B], F32, tag="xa")
                        nc.vector.tensor_add(xa[:], s1[:], x_f[:])
                        if flags['use_bo']:
                            nc.vector.tensor_tensor(out=xa[:], in0=xa[:], in1=bcol(smc(f'boT{i}')),
                                                    op=ALU.add)
                        y_f, y16 = emit_ln(xa, f'ln1wT{i}', f'ln1bT{i}', flags['use_ln1'], f"l1L{i}")
                        x_f, x16 = emit_ffn(i, y_f, y16, f"L{i}")

                    # ---------- logits ----------
                    lg_ps = ps_mm.tile([32, B], F32, tag="mm")
                    for dt in range(ND):
                        nc.tensor.matmul(lg_ps[:], wOutT_sb[:, dt, :], x16[:, dt, :],
                                         start=(dt == 0), stop=(dt == ND - 1))
                    lg_sb = lp_.tile([32, B], F32, tag="lgsb")
                    if flags['use_bout']:
                        nc.vector.tensor_scalar(out=lg_sb[:], in0=lg_ps[:],
                                                scalar1=smc('boutT')[0:32, :], scalar2=None,
                                                op0=ALU.add)
                    else:
                        nc.vector.tensor_copy(lg_sb[:], lg_ps[:])
                    nc.sync.dma_start(out_t[:], lg_sb[:])

    nc.compile()
    return nc


def kernel(**inputs):
    import sys
    if '/opt/trn_rl_repo' not in sys.path:
        sys.path.insert(0, '/opt/trn_rl_repo')
    from concourse.bass_utils import run_bass_kernel_spmd

    in_maps, flags, meta = _prep_inputs(inputs)
    key = tuple(sorted(flags.items())) + (meta['n_smalls'],)
    if key not in _BUILT:
        _BUILT[key] = _build(flags, meta['n_smalls'], meta['smalls'])
    nc = _BUILT[key]

    res = run_bass_kernel_spmd(nc, in_maps, core_ids=list(range(NCORE)))
    logits = np.zeros((B, 1, V), np.float32)
    for c in range(NCORE):
        o = res.results[c]['out']            # [32, B]
        logits[:, 0, c * 32:(c + 1) * 32] = o.T
    return logits


# revision 11
# speedup vs baseline: 19273.0895x; 1.3327x over previous
"""Trainium2 Bass kernel for nn_CachedDecoderOnly (cached decode step).

v3 strategy (8 NeuronCores, SPMD — same NEFF, per-core data):
  - Host precomputes nat = tgt + pe (bf16) in both layouts and
    qk = (q@wq^T+bq) @ wk per head (fp32); bk drops out of softmax.
  - Layer-0 attention: scores = qk @ nat^T per batch on PE; exp on ACT;
    ctx computed TRANSPOSED on PE (no per-batch DMA transposes);
    S sharded over cores, one AllGather + rank-sum combines partial
    (numerator, denominator). Head-batch index hb = h*8+b.
  - Layers 1..5 collapse to x -> M x (M = wo@wv host-folded) -> LN ->
    FFN -> LN. The stack runs tensor-parallel with a "k-pack" activation
    layout (d = p*8+k: partition p, free k) so every collective staging
    and gather DMA moves contiguous >=256B runs.
  - LN-commute: z = M @ LN(r) is computed as s*(M @ r) - s*m*(M@1), so
    the M-matmul + AllGather fly concurrently with the LayerNorm chain.
    Same trick for lin1 (pre-relu rescale) and the final logits.
  - Weight DMAs are issued after the attention loads so attention starts
    immediately; bf16 weights, fp32 PSUM accumulation.
"""

import numpy as np

B, S, D, H, DH, FF, V, L = 8, 4096, 1024, 16, 64, 2048, 256, 6
NCORE = 8
SC = S // NCORE          # 512 sequence rows per core per batch
NT = SC // 128           # 4 s-tiles
ND = D // 128            # 8 d-tiles
NK = 8                   # k-pack factor (d = p*8+k)
SCALE = 1.0 / 8.0        # 1/sqrt(DH)

_BUILT = {}


def _pe_np():
    pos = np.arange(S, dtype=np.float32)[:, None]
    div = np.exp(np.arange(0, D, 2, dtype=np.float32) * (-np.log(10000.0) / D))
    pe = np.zeros((S, D), np.float32)
    pe[:, 0::2] = np.sin(pos * div)
    pe[:, 1::2] = np.cos(pos * div)
    return pe


class _Smalls:
    """Packs per-core [128, n] fp32 constant columns; returns column slices."""

    def __init__(self):
        self.cols = {}
        self.n = 0

    def add(self, name, arr):  # arr [128, k]
        arr = np.asarray(arr, np.float32)
        assert arr.shape[0] == 128
        k = arr.shape[1] if arr.ndim == 2 else 1
        arr = arr.reshape(128, k)
        self.cols[name] = (self.n, k, arr)
        self.n += k

    def pack(self):
        out = np.zeros((128, max(self.n, 1)), np.float32)
        for off, k, arr in self.cols.values():
            out[:, off:off + k] = arr
        return out


def _vec_KP(v):
    """[D] fp32 -> [128, NK] k-pack: out[p, k] = v[p*8+k]."""
    return np.ascontiguousarray(np.asarray(v, np.float32).reshape(128, NK))


def _prep_inputs(inputs):
    """Host-side weight re-layout + sharding. Returns (in_maps, flags, meta)."""
    f32 = np.float32
    g = {k: np.asarray(v) for k, v in inputs.items()}
    pe = _pe_np()

    in_proj_w = g['in_proj_w'].astype(f32)
    in_proj_b = g['in_proj_b'].astype(f32)
    out_proj_w = g['out_proj_w'].astype(f32)
    out_proj_b = g['out_proj_b'].astype(f32)
    lin1_w, lin1_b = g['lin1_w'].astype(f32), g['lin1_b'].astype(f32)
    lin2_w, lin2_b = g['lin2_w'].astype(f32), g['lin2_b'].astype(f32)
    ln1_w, ln1_b = g['ln1_w'].astype(f32), g['ln1_b'].astype(f32)
    ln2_w, ln2_b = g['ln2_w'].astype(f32), g['ln2_b'].astype(f32)
    w_out, b_out = g['w_out'].astype(f32), g['b_out'].astype(f32)
    tgt = g['tgt'].astype(f32)

    flags = {
        'use_bv': bool(np.any(in_proj_b[:, 2 * D:] != 0)),
        'use_bo': bool(np.any(out_proj_b != 0)),
        'use_b1': bool(np.any(lin1_b != 0)),
        'use_b2': bool(np.any(lin2_b != 0)),
        'use_bout': bool(np.any(b_out != 0)),
        'use_ln1': bool(np.any(ln1_w != 1) or np.any(ln1_b != 0)),
        'use_ln2': bool(np.any(ln2_w != 1) or np.any(ln2_b != 0)),
    }

    import ml_dtypes
    bf16 = ml_dtypes.bfloat16

    # host-folded q/k projections (fp32): scores = qk . nat
    q = tgt[:, -1, :] + pe[-1]                                   # [B, D]
    Q = q @ in_proj_w[0, 0:D].T + in_proj_b[0, 0:D]
    wk = in_proj_w[0, D:2 * D]
    qk = np.zeros((B, H, D), f32)
    for h in range(H):
        qk[:, h, :] = Q[:, h * DH:(h + 1) * DH] @ wk[h * DH:(h + 1) * DH, :]
    qkT = np.ascontiguousarray(qk.transpose(2, 0, 1).reshape(ND, 128, B, H)
                               .transpose(1, 0, 2, 3)).astype(bf16)
    qkp = np.ascontiguousarray(q.T.reshape(128, NK, B))          # residual, k-pack f32

    wv0 = in_proj_w[0, 2 * D:]
    wvT0 = np.ascontiguousarray(wv0.T.reshape(ND, 128, D).transpose(1, 0, 2)).astype(bf16)

    # layers 1..5: M = wo @ wv, with previous layer's ln2 affine folded in
    Ms, cOffM = {}, {}
    for i in range(1, L):
        M = out_proj_w[i] @ in_proj_w[i, 2 * D:]                 # [D, D]
        cOff = np.zeros(D, f32)
        if flags['use_bv']:
            cOff += out_proj_w[i] @ in_proj_b[i, 2 * D:]
        if flags['use_bo']:
            cOff += out_proj_b[i]
        if flags['use_ln2']:
            cOff += M @ ln2_b[i - 1]
            M = M * ln2_w[i - 1][None, :]
        Ms[i], cOffM[i] = M, cOff

    # lin1 with own-layer ln1 affine folded
    lin1f, cOff1 = [], []
    for i in range(L):
        l1 = lin1_w[i]
        cO = np.zeros(FF, f32)
        if flags['use_ln1']:
            cO += l1 @ ln1_b[i]
            l1 = l1 * ln1_w[i][None, :]
        lin1f.append(l1)
        cOff1.append(cO)

    # w_out with layer-5 ln2 affine folded
    wOutf = w_out
    cOffOut = np.zeros(V, f32)
    if flags['use_ln2']:
        cOffOut += w_out @ ln2_b[L - 1]
        wOutf = w_out * ln2_w[L - 1][None, :]
    if flags['use_bout']:
        cOffOut += b_out

    shared = {'qkT': qkT, 'qkp': qkp, 'wvT0': wvT0}

    in_maps = []
    for c in range(NCORE):
        m = dict(shared)
        natc = (tgt[:, c * SC:(c + 1) * SC, :] + pe[c * SC:(c + 1) * SC]).astype(bf16)
        m['natC'] = np.ascontiguousarray(
            natc.reshape(B, NT, 128, D).transpose(0, 2, 1, 3))
        m['natT'] = np.ascontiguousarray(
            natc.transpose(0, 2, 1).reshape(B, ND, 128, SC).transpose(0, 2, 1, 3))
        # woT0k [128, NK, 128]: [e_l, k, p] = wo0[p*8+k, c*128+e_l]
        m['woT0k'] = np.ascontiguousarray(
            out_proj_w[0].T[c * 128:(c + 1) * 128].reshape(128, 128, NK)
            .transpose(0, 2, 1)).astype(bf16)
        for i in range(1, L):
            # MT [128, NK, 128]: [p, k, o_l] = M[c*128+o_l, p*8+k]
            m[f'MT{i}'] = np.ascontiguousarray(
                Ms[i].T[:, c * 128:(c + 1) * 128].reshape(128, NK, 128)).astype(bf16)
        for i in range(L):
            # w1T [128, NK, 2, 128]: [p, k, fs, fl] = lin1f[c*256+fs*128+fl, p*8+k]
            m[f'w1T{i}'] = np.ascontiguousarray(
                lin1f[i].T[:, c * 256:(c + 1) * 256].reshape(128, NK, 2, 128)).astype(bf16)
            # w2T [128, 2, NK, 128]: [fp, fs, k, p] = lin2[p*8+k, c*256+fs*128+fp]
            m[f'w2T{i}'] = np.ascontiguousarray(
                lin2_w[i].T[c * 256:(c + 1) * 256, :].reshape(2, 128, 128, NK)
                .transpose(1, 0, 3, 2)).astype(bf16)
        # wOutTk [128, NK, 32]: [p, k, vl] = wOutf[c*32+vl, p*8+k]
        m['wOutTk'] = np.ascontiguousarray(
            wOutf.T[:, c * 32:(c + 1) * 32].reshape(128, NK, 32)).astype(bf16)

        sm = _Smalls()
        sm.add('id8', np.vstack([np.eye(8, dtype=f32), np.zeros((120, 8), f32)]))
        # den relayout (hb = h*8+b): sel[h, p] = 1 iff p//8 == h ; mask[p, b] = 1 iff p%8 == b
        sel = np.zeros((128, 128), f32)
        for p in range(128):
            sel[p // 8, p] = 1.0
        sm.add('sel', sel)
        mask = np.zeros((128, 8), f32)
        for p in range(128):
            mask[p, p % 8] = 1.0
        sm.add('mask', mask)
        # per-core head-pair select: osel[hb, m] = 1 iff hb == (2c + m//8)*8 + m%8
        osel = np.zeros((128, 16), f32)
        for mm_ in range(16):
            osel[(2 * c + mm_ // 8) * 8 + (mm_ % 8), mm_] = 1.0
        sm.add('osel', osel)
        for i in range(1, L):
            sm.add(f'csM{i}', _vec_KP(Ms[i] @ np.ones(D, f32)))
            if np.any(cOffM[i] != 0):
                sm.add(f'cOffM{i}', _vec_KP(cOffM[i]))
        for i in range(L):
            cs1 = lin1f[i][c * 256:(c + 1) * 256] @ np.ones(D, f32)   # [256]
            sm.add(f'cs1_{i}', cs1.reshape(2, 128).T)                 # [128(fl), 2(fs)]
            if np.any(cOff1[i] != 0):
                sm.add(f'cOff1_{i}',
                       cOff1[i][c * 256:(c + 1) * 256].reshape(2, 128).T)
        csOut = np.zeros((128, 1), f32)
        csOut[0:32, 0] = wOutf[c * 32:(c + 1) * 32] @ np.ones(D, f32)
        sm.add('csOut', csOut)
        if np.any(cOffOut != 0):
            co = np.zeros((128, 1), f32)
            co[0:32, 0] = cOffOut[c * 32:(c + 1) * 32]
            sm.add('cOffOut', co)
        if flags['use_bv']:
            bv0 = in_proj_b[0, 2 * D:]
            bvexp = np.zeros((128, 64), f32)
            for p in range(128):
                bvexp[p, :] = bv0[(p // 8) * 64:(p // 8) * 64 + 64]
            sm.add('bvExp0', bvexp)
        if flags['use_bo']:
            sm.add('boT0k', _vec_KP(out_proj_b[0]))
        if flags['use_b1']:
            for i in range(L):
                sm.add(f'b1T{i}', lin1_b[i, c * 256:(c + 1) * 256].reshape(2, 128).T)
        if flags['use_b2']:
            for i in range(L):
                sm.add(f'b2T{i}', _vec_KP(lin2_b[i]))
        if flags['use_ln1']:
            for i in range(L):
                sm.add(f'ln1wK{i}', _vec_KP(ln1_w[i]))
                sm.add(f'ln1bK{i}', _vec_KP(ln1_b[i]))
        if flags['use_ln2']:
            for i in range(L - 1):
                sm.add(f'ln2wK{i}', _vec_KP(ln2_w[i]))
                sm.add(f'ln2bK{i}', _vec_KP(ln2_b[i]))
        m['smalls'] = sm.pack()
        in_maps.append(m)

    meta = {'smalls': sm, 'n_smalls': in_maps[0]['smalls'].shape[1]}
    return in_maps, flags, meta


def _build(flags, n_smalls, smalls_obj):
    import concourse.bass as bass
    import concourse.tile as tile
    import concourse.mybir as mybir
    from concourse import bacc

    F32 = mybir.dt.float32
    BF16 = mybir.dt.bfloat16
    AF = mybir.ActivationFunctionType
    ALU = mybir.AluOpType

    nc = bacc.Bacc("TRN2", target_bir_lowering=False, num_devices=NCORE)

    def din(name, shape, dt=F32):
        return nc.dram_tensor(name, shape, dt, kind="ExternalInput")

    natC_in = din('natC', [B, 128, NT, D], BF16)
    natT_in = din('natT', [B, 128, ND, SC], BF16)
    qkT_in = din('qkT', [128, ND, B, 16], BF16)
    qkp_in = din('qkp', [128, NK, B])
    wvT0_in = din('wvT0', [128, ND, D], BF16)
    woT0k_in = din('woT0k', [128, NK, 128], BF16)
    MT_in = {i: din(f'MT{i}', [128, NK, 128], BF16) for i in range(1, L)}
    w1T_in = [din(f'w1T{i}', [128, NK, 2, 128], BF16) for i in range(L)]
    w2T_in = [din(f'w2T{i}', [128, 2, NK, 128], BF16) for i in range(L)]
    wOutTk_in = din('wOutTk', [128, NK, 32], BF16)
    smalls_in = din('smalls', [128, n_smalls])
    out_t = nc.dram_tensor('out', [32, B], F32, kind="ExternalOutput")

    RG = [list(range(NCORE))]

    with tile.TileContext(nc) as tc:
        with (
            tc.tile_pool(name="persist", bufs=1) as pp,
            tc.tile_pool(name="state", bufs=2) as stp,
            tc.tile_pool(name="dram", bufs=3, space="DRAM") as dram,
            tc.tile_pool(name="dram_sh", bufs=3, space="DRAM") as dram_sh,
        ):
            sm = pp.tile([128, n_smalls], F32)
            nc.sync.dma_start(sm[:], smalls_in[:])

            def smc(name):
                off, k, _ = smalls_obj.cols[name]
                return sm[:, off:off + k]

            def has(name):
                return name in smalls_obj.cols

            ones128 = pp.tile([128, 1], F32)
            nc.vector.memset(ones128[:], 1.0)
            ones8w = pp.tile([8, 128], F32)
            nc.vector.memset(ones8w[:], 1.0)
            eps8 = pp.tile([8, 1], F32)
            nc.vector.memset(eps8[:], 1e-5)

            qkT_sb = pp.tile([128, ND, B, 16], BF16)
            nc.sync.dma_start(qkT_sb[:], qkT_in[:])
            qkp_t = pp.tile([128, NK, B], F32)
            nc.sync.dma_start(qkp_t[:], qkp_in[:])

            den_stack = pp.tile([16, B], F32)
            ctxT = pp.tile([128, ND, 128], BF16)     # [e_l, et, hb], hb = h*8+b

            # ---------- attention (S-sharded), per batch ----------
            with (
                tc.tile_pool(name="attn", bufs=3) as ap_,
                tc.tile_pool(name="ps_sc", bufs=2, space="PSUM") as ps_sc,
                tc.tile_pool(name="ps_ctx", bufs=2, space="PSUM") as ps_ctx,
            ):
                with nc.named_scope("attn"):
                    for b in range(B):
                        natT_b = ap_.tile([128, ND, SC], BF16, tag="natT")
                        nc.sync.dma_start(natT_b[:], natT_in[b])
                        natC_b = ap_.tile([128, NT, D], BF16, tag="natC")
                        nc.sync.dma_start(natC_b[:], natC_in[b])
                        sc_ps = ps_sc.tile([16, SC], F32, tag="sc")
                        for dt in range(ND):
                            nc.tensor.matmul(sc_ps[:], qkT_sb[:, dt, b, :], natT_b[:, dt, :],
                                             start=(dt == 0), stop=(dt == ND - 1))
                        a16 = ap_.tile([16, SC], BF16, tag="a16")
                        nc.scalar.activation(out=a16[:], in_=sc_ps[:], func=AF.Exp,
                                             scale=SCALE, accum_out=den_stack[:, b:b + 1])
                        aT = ap_.tile([128, NT, 16], BF16, tag="aT")
                        nc.scalar.dma_start_transpose(aT[:], a16[:])
                        ctxT_ps = ps_ctx.tile([128, ND, 16], F32, tag="ctx")
                        for et in range(ND):
                            for st in range(NT):
                                nc.tensor.matmul(ctxT_ps[:, et, :],
                                                 natC_b[:, st, et * 128:(et + 1) * 128],
                                                 aT[:, st, :],
                                                 start=(st == 0), stop=(st == NT - 1))
                        nc.vector.tensor_copy(ctxT[:, :, b:b + 121:8], ctxT_ps[:])

            # ---------- weight preloads (issued after attention loads) ----------
            wvT0_sb = pp.tile([128, ND, D], BF16)
            nc.sync.dma_start(wvT0_sb[:], wvT0_in[:])
            woT0k_sb = pp.tile([128, NK, 128], BF16)
            nc.sync.dma_start(woT0k_sb[:], woT0k_in[:])
            w1T_sb, w2T_sb, MT_sb = [], [], {}
            for i in range(L):
                t1 = pp.tile([128, NK, 2, 128], BF16, name=f'w1Ts{i}')
                nc.sync.dma_start(t1[:], w1T_in[i][:])
                w1T_sb.append(t1)
                t2 = pp.tile([128, 2, NK, 128], BF16, name=f'w2Ts{i}')
                nc.sync.dma_start(t2[:], w2T_in[i][:])
                w2T_sb.append(t2)
                if i >= 1:
                    tm = pp.tile([128, NK, 128], BF16, name=f'MTs{i}')
                    nc.sync.dma_start(tm[:], MT_in[i][:])
                    MT_sb[i] = tm
            wOutTk_sb = pp.tile([128, NK, 32], BF16)
            nc.sync.dma_start(wOutTk_sb[:], wOutTk_in[:])

            # ---------- u = ctx @ wv0^T, diag, den ----------
            with (
                tc.tile_pool(name="upool", bufs=1) as up_,
                tc.tile_pool(name="ps_u", bufs=1, space="PSUM") as ps_u,
                tc.tile_pool(name="ps_us", bufs=1, space="PSUM") as ps_us,
            ):
                with nc.named_scope("u"):
                    u_ps = ps_u.tile([128, D], F32, tag="u")
                    for et in range(ND):
                        for hf in range(2):
                            nc.tensor.matmul(u_ps[:, hf * 512:(hf + 1) * 512],
                                             ctxT[:, et, :], wvT0_sb[:, et, hf * 512:(hf + 1) * 512],
                                             start=(et == 0), stop=(et == ND - 1))
                    u_f = up_.tile([128, D], F32)
                    nc.vector.tensor_copy(u_f[:], u_ps[:])
                    u_sb = up_.tile([128, 65], F32)
                    for h in range(H):
                        eng = nc.sync if h % 2 == 0 else nc.scalar
                        eng.dma_start(u_sb[h * 8:(h + 1) * 8, 0:64],
                                      u_f[h * 8:(h + 1) * 8, h * 64:(h + 1) * 64])
                    dsel_ps = ps_us.tile([128, 8], F32, tag="dsel")
                    nc.tensor.matmul(dsel_ps[:], smc('sel')[0:16, :], den_stack[:],
                                     start=True, stop=True)
                    dmask = up_.tile([128, 8], F32)
                    nc.vector.tensor_mul(dmask[:], dsel_ps[:], smc('mask'))
                    nc.vector.tensor_reduce(u_sb[:, 64:65], dmask[:],
                                            axis=mybir.AxisListType.X, op=ALU.add)

                # ---------- u-sync (AllGather + rank-sum) ----------
                with nc.named_scope("usync"):
                    cc1_in = dram.tile([128, 65], F32, tag="cc1i")
                    cc1_out = dram_sh.tile([128 * NCORE, 65], F32, addr_space="Shared", tag="cc1o")
                    nc.sync.dma_start(cc1_in[:], u_sb[:])
                    nc.gpsimd.collective_compute("AllGather", mybir.AluOpType.bypass,
                                                 replica_groups=RG,
                                                 ins=[cc1_in.opt()], outs=[cc1_out.opt()])
                    g1 = stp.tile([128, NCORE, 65], F32, tag="g1")
                    nc.sync.dma_start(g1[:], cc1_out[:].rearrange("(r p) x -> p r x", r=NCORE))
                    t4 = stp.tile([128, 4, 65], F32, tag="t4")
                    nc.vector.tensor_add(t4[:], g1[:, 0:4, :], g1[:, 4:8, :])
                    t2 = stp.tile([128, 2, 65], F32, tag="t2")
                    nc.vector.tensor_add(t2[:], t4[:, 0:2, :], t4[:, 2:4, :])
                    t1 = stp.tile([128, 65], F32, tag="t1")
                    nc.vector.tensor_add(t1[:], t2[:, 0, :], t2[:, 1, :])
                    rd = stp.tile([128, 1], F32, tag="rd")
                    nc.vector.reciprocal(rd[:], t1[:, 64:65])
                    o16 = stp.tile([128, 64], F32, tag="o16")
                    nc.vector.tensor_scalar_mul(o16[:], t1[:, 0:64], rd[:])
                    if flags['use_bv']:
                        nc.vector.tensor_tensor(out=o16[:], in0=o16[:],
                                                in1=smc('bvExp0'), op=ALU.add)
                    os_ps = ps_us.tile([16, 64], F32, tag="osps")
                    nc.tensor.matmul(os_ps[:], smc('osel'), o16[:], start=True, stop=True)
                    os16 = stp.tile([16, 128], BF16, tag="os16")
                    nc.vector.memset(os16[:, 64:128], 0.0)
                    nc.vector.tensor_copy(os16[:, 0:64], os_ps[:])
                    xoc = stp.tile([128, 16], BF16, tag="xoc")
                    nc.scalar.dma_start_transpose(xoc[:], os16[:])
                    u_c = stp.tile([128, B], BF16, tag="uc")
                    nc.gpsimd.dma_start(u_c[0:64, :], xoc[0:64, 0:8])
                    nc.gpsimd.dma_start(u_c[64:128, :], xoc[0:64, 8:16])

            # ---------- sequential layer stack (k-pack layout) ----------
            with (
                tc.tile_pool(name="lay", bufs=2) as lp_,
                tc.tile_pool(name="ps_mm", bufs=2, space="PSUM") as ps_mm,
                tc.tile_pool(name="ps_ln", bufs=1, space="PSUM") as ps_ln,
            ):
                def bcol(col):
                    """[128, NK] per-k column -> broadcast AP [128, NK, B]."""
                    return col.rearrange("p (nk a) -> p nk a", a=1).to_broadcast((128, NK, B))

                def bcrow(row, nd=NK):
                    """[128, 8] per-b row -> broadcast AP [128, nd, B]."""
                    return row.rearrange("p (a b) -> p a b", a=1).to_broadcast((128, nd, B))

                def emit_ln_stats(x_f, nm):
                    """LN stats of [128, NK, B] over d -> bc_sb [128, 16]:
                    cols 0:8 = rstd_b, cols 8:16 = mean_b*rstd_b (partition-bcast)."""
                    st_ps = ps_ln.tile([8, 9], F32, tag="st")
                    for k in range(NK):
                        nc.tensor.matmul(st_ps[:, 0:1], x_f[:, k, :], ones128[:],
                                         start=(k == 0), stop=(k == NK - 1))
                        nc.tensor.matmul(st_ps[:, 1:9], x_f[:, k, :], x_f[:, k, :],
                                         start=(k == 0), stop=(k == NK - 1))
                    gsb = lp_.tile([8, 9], F32, tag="gsb")
                    nc.vector.tensor_copy(gsb[:], st_ps[:])
                    gm = lp_.tile([8, 8], F32, tag="gm")
                    nc.vector.tensor_mul(gm[:], gsb[:, 1:9], smc('id8')[0:8, :])
                    mv = lp_.tile([8, 4], F32, tag="mv")  # ss, m, var, mr
                    nc.vector.tensor_reduce(mv[:, 0:1], gm[:], axis=mybir.AxisListType.X, op=ALU.add)
                    nc.vector.tensor_scalar_mul(mv[:, 1:2], gsb[:, 0:1], 1.0 / D)
                    nc.vector.tensor_scalar_mul(mv[:, 0:1], mv[:, 0:1], 1.0 / D)
                    m2 = lp_.tile([8, 1], F32, tag="m2")
                    nc.vector.tensor_mul(m2[:], mv[:, 1:2], mv[:, 1:2])
                    nc.vector.tensor_sub(mv[:, 2:3], mv[:, 0:1], m2[:])
                    sq = lp_.tile([8, 1], F32, tag="sq")
                    nc.scalar.activation(out=sq[:], in_=mv[:, 2:3], func=AF.Sqrt, bias=eps8[:])
                    rstd = lp_.tile([8, 1], F32, tag="rstd")
                    nc.vector.reciprocal(rstd[:], sq[:])
                    nc.vector.tensor_mul(mv[:, 3:4], mv[:, 1:2], rstd[:])
                    dg = lp_.tile([8, 16], F32, tag="dg")
                    nc.vector.tensor_scalar_mul(dg[:, 0:8], smc('id8')[0:8, :], rstd[:])
                    nc.vector.tensor_scalar_mul(dg[:, 8:16], smc('id8')[0:8, :], mv[:, 3:4])
                    bc_ps = ps_ln.tile([128, 16], F32, tag="bc")
                    nc.tensor.matmul(bc_ps[:], ones8w[:], dg[:], start=True, stop=True)
                    bc_sb = lp_.tile([128, 16], F32, tag=f"bc{nm}")
                    nc.vector.tensor_copy(bc_sb[:], bc_ps[:])
                    return bc_sb

                def ffn_leg(i, xa, nm):
                    """xa [128, NK, B] pre-LN1 raw -> (r_f, r16) pre-LN2 raw.
                    lin1-commute: lin1 @ LN1(xa) = s'(lin1' xa) - s'm'(lin1' 1) [+ lin1 b1ln]."""
                    xa16 = lp_.tile([128, NK, B], BF16, tag="xa16")
                    nc.vector.tensor_copy(xa16[:], xa[:])
                    q_ps = ps_mm.tile([128, 2, B], F32, tag="mm")
                    for fs in range(2):
                        for k in range(NK):
                            nc.tensor.matmul(q_ps[:, fs, :], w1T_sb[i][:, k, fs, :],
                                             xa16[:, k, :], start=(k == 0), stop=(k == NK - 1))
                    bc1p = emit_ln_stats(xa, f"l1{nm}")
                    hc = lp_.tile([128, 2, B], F32, tag="hc")
                    for fs in range(2):
                        nc.vector.tensor_scalar(out=hc[:, fs, :], in0=bc1p[:, 8:16],
                                                scalar1=smc(f'cs1_{i}')[:, fs:fs + 1],
                                                scalar2=None, op0=ALU.mult)
                    hs = lp_.tile([128, 2, B], F32, tag="hs")
                    nc.vector.tensor_tensor(out=hs[:], in0=q_ps[:], in1=bcrow(bc1p[:, 0:8], 2),
                                            op=ALU.mult)
                    nc.vector.tensor_sub(hs[:], hs[:], hc[:])
                    if has(f'cOff1_{i}'):
                        for fs in range(2):
                            nc.vector.tensor_scalar(out=hs[:, fs, :], in0=hs[:, fs, :],
                                                    scalar1=smc(f'cOff1_{i}')[:, fs:fs + 1],
                                                    scalar2=None, op0=ALU.add)
                    if flags['use_b1']:
                        for fs in range(2):
                            nc.vector.tensor_scalar(out=hs[:, fs, :], in0=hs[:, fs, :],
                                                    scalar1=smc(f'b1T{i}')[:, fs:fs + 1],
                                                    scalar2=None, op0=ALU.add)
                    h16 = lp_.tile([128, 2, B], BF16, tag="h16")
                    nc.vector.tensor_scalar(out=h16[:], in0=hs[:], scalar1=0.0,
                                            scalar2=None, op0=ALU.max)
                    ff_ps = ps_mm.tile([128, NK, B], F32, tag="mm")
                    for k in range(NK):
                        for fs in range(2):
                            nc.tensor.matmul(ff_ps[:, k, :], w2T_sb[i][:, fs, k, :],
                                             h16[:, fs, :], start=(fs == 0), stop=(fs == 1))
                    ff_sb = lp_.tile([128, NK, B], F32, tag="ffsb")
                    nc.vector.tensor_copy(ff_sb[:], ff_ps[:])
                    pin = dram.tile([128, NK * B], F32, tag="pin")
                    pout = dram_sh.tile([128 * NCORE, NK * B], F32, addr_space="Shared", tag="pout")
                    nc.sync.dma_start(pin[:], ff_sb[:].rearrange("p nk b -> p (nk b)"))
                    nc.gpsimd.collective_compute("AllGather", mybir.AluOpType.bypass,
                                                 replica_groups=RG,
                                                 ins=[pin.opt()], outs=[pout.opt()])
                    gg = lp_.tile([128, NCORE, NK, B], F32, tag="gg")
                    nc.sync.dma_start(gg[:], pout[:].rearrange("(r p) (nk b) -> p r nk b",
                                                               r=NCORE, nk=NK))
                    # y1 = LN1(xa) materialized for the residual
                    y1 = lp_.tile([128, NK, B], F32, tag=f"y1{nm}")
                    nc.vector.tensor_tensor(out=y1[:], in0=xa[:], in1=bcrow(bc1p[:, 0:8]),
                                            op=ALU.mult)
                    nc.vector.tensor_tensor(out=y1[:], in0=y1[:], in1=bcrow(bc1p[:, 8:16]),
                                            op=ALU.subtract)
                    if flags['use_ln1']:
                        nc.vector.tensor_tensor(out=y1[:], in0=y1[:], in1=bcol(smc(f'ln1wK{i}')),
                                                op=ALU.mult)
                        nc.vector.tensor_tensor(out=y1[:], in0=y1[:], in1=bcol(smc(f'ln1bK{i}')),
                                                op=ALU.add)
                    s4 = lp_.tile([128, 4, NK, B], F32, tag="s4")
                    nc.vector.tensor_add(s4[:], gg[:, 0:4], gg[:, 4:8])
                    s2 = lp_.tile([128, 2, NK, B], F32, tag="s2")
                    nc.vector.tensor_add(s2[:], s4[:, 0:2], s4[:, 2:4])
                    r_f = lp_.tile([128, NK, B], F32, tag=f"rf{nm}")
                    nc.vector.tensor_add(r_f[:], s2[:, 0], s2[:, 1])
                    nc.vector.tensor_add(r_f[:], r_f[:], y1[:])
                    if flags['use_b2']:
                        nc.vector.tensor_tensor(out=r_f[:], in0=r_f[:], in1=bcol(smc(f'b2T{i}')),
                                                op=ALU.add)
                    r16 = lp_.tile([128, NK, B], BF16, tag=f"r16{nm}")
                    nc.vector.tensor_copy(r16[:], r_f[:])
                    return r_f, r16

                with nc.named_scope("stack"):
                    # ----- layer 0 out_proj partial (e-sharded, k-pack out) -----
                    z_ps = ps_mm.tile([128, NK, B], F32, tag="mm")
                    for k in range(NK):
                        nc.tensor.matmul(z_ps[:, k, :], woT0k_sb[:, k, :], u_c[:],
                                         start=True, stop=True)
                    z_sb = lp_.tile([128, NK, B], F32, tag="zsb0")
                    nc.vector.tensor_copy(z_sb[:], z_ps[:])
                    pin0 = dram.tile([128, NK * B], F32, tag="pin")
                    pout0 = dram_sh.tile([128 * NCORE, NK * B], F32, addr_space="Shared", tag="pout")
                    nc.sync.dma_start(pin0[:], z_sb[:].rearrange("p nk b -> p (nk b)"))
                    nc.gpsimd.collective_compute("AllGather", mybir.AluOpType.bypass,
                                                 replica_groups=RG,
                                                 ins=[pin0.opt()], outs=[pout0.opt()])
                    gg0 = lp_.tile([128, NCORE, NK, B], F32, tag="gg")
                    nc.sync.dma_start(gg0[:], pout0[:].rearrange("(r p) (nk b) -> p r nk b",
                                                                 r=NCORE, nk=NK))
                    s4 = lp_.tile([128, 4, NK, B], F32, tag="s4")
                    nc.vector.tensor_add(s4[:], gg0[:, 0:4], gg0[:, 4:8])
                    s2 = lp_.tile([128, 2, NK, B], F32, tag="s2")
                    nc.vector.tensor_add(s2[:], s4[:, 0:2], s4[:, 2:4])
                    x1 = lp_.tile([128, NK, B], F32, tag="x1l0")
                    nc.vector.tensor_add(x1[:], s2[:, 0], s2[:, 1])
                    nc.vector.tensor_add(x1[:], x1[:], qkp_t[:])
                    if flags['use_bo']:
                        nc.vector.tensor_tensor(out=x1[:], in0=x1[:], in1=bcol(smc('boT0k')),
                                                op=ALU.add)
                    r_f, r16 = ffn_leg(0, x1, "L0")

                    # ----- layers 1..5: z-leg (M-fold + LN2-commute), then ffn-leg -----
                    for i in range(1, L):
                        p_ps = ps_mm.tile([128, B], F32, tag="mm")
                        for k in range(NK):
                            nc.tensor.matmul(p_ps[:], MT_sb[i][:, k, :], r16[:, k, :],
                                             start=(k == 0), stop=(k == NK - 1))
                        p_sb = lp_.tile([128, B], F32, tag="psb")
                        nc.vector.tensor_copy(p_sb[:], p_ps[:])
                        pinz = dram.tile([128, B], F32, tag="pinz")
                        poutz = dram_sh.tile([128 * NCORE, B], F32, addr_space="Shared", tag="poutz")
                        nc.sync.dma_start(pinz[:], p_sb[:])
                        nc.gpsimd.collective_compute("AllGather", mybir.AluOpType.bypass,
                                                     replica_groups=RG,
                                                     ins=[pinz.opt()], outs=[poutz.opt()])
                        bc2 = emit_ln_stats(r_f, f"l2L{i}")
                        # x = LN2(r) materialized for the residual
                        x_f = lp_.tile([128, NK, B], F32, tag="xf")
                        nc.vector.tensor_tensor(out=x_f[:], in0=r_f[:], in1=bcrow(bc2[:, 0:8]),
                                                op=ALU.mult)
                        nc.vector.tensor_tensor(out=x_f[:], in0=x_f[:], in1=bcrow(bc2[:, 8:16]),
                                                op=ALU.subtract)
                        if flags['use_ln2']:
                            nc.vector.tensor_tensor(out=x_f[:], in0=x_f[:],
                                                    in1=bcol(smc(f'ln2wK{i-1}')), op=ALU.mult)
                            nc.vector.tensor_tensor(out=x_f[:], in0=x_f[:],
                                                    in1=bcol(smc(f'ln2bK{i-1}')), op=ALU.add)
                        # t2 = m*rstd * colsumM (per-k), during AG flight
                        t2k = lp_.tile([128, NK, B], F32, tag="t2k")
                        for k in range(NK):
                            nc.vector.tensor_scalar(out=t2k[:, k, :], in0=bc2[:, 8:16],
                                                    scalar1=smc(f'csM{i}')[:, k:k + 1],
                                                    scalar2=None, op0=ALU.mult)
                        gz = lp_.tile([128, NK, B], F32, tag="gz")
                        nc.sync.dma_start(gz[:], poutz[:].rearrange("(p nk) b -> p nk b", nk=NK))
                        xa = lp_.tile([128, NK, B], F32, tag="xal")
                        nc.vector.tensor_tensor(out=xa[:], in0=gz[:], in1=bcrow(bc2[:, 0:8]),
                                                op=ALU.mult)
                        nc.vector.tensor_sub(xa[:], xa[:], t2k[:])
                        nc.vector.tensor_add(xa[:], xa[:], x_f[:])
                        if has(f'cOffM{i}'):
                            nc.vector.tensor_tensor(out=xa[:], in0=xa[:],
                                                    in1=bcol(smc(f'cOffM{i}')), op=ALU.add)
                        r_f, r16 = ffn_leg(i, xa, f"L{i}")

                    # ----- logits with LN2-commute -----
                    lg_ps = ps_mm.tile([32, B], F32, tag="mm")
                    for k in range(NK):
                        nc.tensor.matmul(lg_ps[:], wOutTk_sb[:, k, :], r16[:, k, :],
                                         start=(k == 0), stop=(k == NK - 1))
                    bcO = emit_ln_stats(r_f, "lO")
                    lg_sb = lp_.tile([32, B], F32, tag="lgsb")
                    nc.vector.tensor_mul(lg_sb[:], lg_ps[:], bcO[0:32, 0:8])
                    lgc = lp_.tile([32, B], F32, tag="lgc")
                    nc.vector.tensor_scalar(out=lgc[:], in0=bcO[0:32, 8:16],
                                            scalar1=smc('csOut')[0:32, :],
                                            scalar2=None, op0=ALU.mult)
                    nc.vector.tensor_sub(lg_sb[:], lg_sb[:], lgc[:])
                    if has('cOffOut'):
                        nc.vector.tensor_scalar(out=lg_sb[:], in0=lg_sb[:],
                                                scalar1=smc('cOffOut')[0:32, :],
                                                scalar2=None, op0=ALU.add)
                    nc.sync.dma_start(out_t[:], lg_sb[:])

    nc.compile()
    return nc


def kernel(**inputs):
    import sys
    if '/opt/trn_rl_repo' not in sys.path:
        sys.path.insert(0, '/opt/trn_rl_repo')
    from concourse.bass_utils import run_bass_kernel_spmd

    in_maps, flags, meta = _prep_inputs(inputs)
    key = tuple(sorted(flags.items())) + (meta['n_smalls'],)
    if key not in _BUILT:
        _BUILT[key] = _build(flags, meta['n_smalls'], meta['smalls'])
    nc = _BUILT[key]

    res = run_bass_kernel_spmd(nc, in_maps, core_ids=list(range(NCORE)))
    logits = np.zeros((B, 1, V), np.float32)
    for c in range(NCORE):
        o = res.results[c]['out']            # [32, B]
        logits[:, 0, c * 32:(c + 1) * 32] = o.T
    return logits
